# revision 1
# baseline (speedup 1.0000x reference)
"""GCN message-passing kernel for Trainium2, 8 NeuronCores.

Math (reference): 3-layer GCN with symmetric normalization and self-loops,
then dot-product decode over label edge pairs.

Key reformulation: A_hat @ (x @ W) == (A_hat @ x) @ W, so each layer is
  agg = A_hat @ x          (sparse gather + scatter)
  z   = relu(agg @ W + b)
A_hat is the same for all 3 layers. All normalization (dinv[src]*dinv[dst],
self-loop dinv^2) is folded into per-edge values.

Device mapping per core (owns 49 consecutive node blocks of 128):
  - edges partitioned by dst block, sorted+chunked into 128-edge chunks
  - dma_gather pulls x[src] rows into SBUF [128 slots, chunks, 128 feat]
    (int16 indices; >=32768 handled by a second gather with the table
    offset by 32768 rows)
  - per chunk, DVE builds indicator[slot, node] = (iota==dstlocal)*norm
    in one tensor_scalar op
  - PE accumulates psum[feat, node] += gathered[slot, feat].T @ indicator
  - per block: z[node, outf] = relu(aggT.T @ W + b) via two matmuls
    (bias via rank-1 ones x bias matmul into the accumulation group)
  - AllGather assembles the full z for the next layer's gathers
Decode: labels bucketed by (a<32768, b<32768); per bucket dma_gather of
z3[a] and z3[b] rows, DVE multiply + reduce, host inverse-permutes.
"""

import numpy as np

P = 128
HALF = 32768
N_CORES = 8


# ---------------------------------------------------------------- host prep

def _wrap16(flat_idx):
    """dma_gather idx layout: idx i at [i%16, i//16], replicated to 128 rows."""
    t = flat_idx.astype(np.int16).reshape(-1, 16).T  # [16, n/16]
    return np.tile(t, (8, 1))  # [128, n/16]


def prepare_edges(edge_index, n_nodes, bpc):
    """Build per-core gather/indicator streams.

    Returns dict with per-core arrays and uniform per-block chunk counts.
    """
    src = np.asarray(edge_index[0], dtype=np.int64)
    dst = np.asarray(edge_index[1], dtype=np.int64)
    deg = np.bincount(dst, minlength=n_nodes).astype(np.float64) + 1.0
    dinv = 1.0 / np.sqrt(deg)

    # full edge list incl self-loops, with folded normalization values
    loops = np.arange(n_nodes, dtype=np.int64)
    esrc = np.concatenate([src, loops])
    edst = np.concatenate([dst, loops])
    enrm = np.concatenate([dinv[src] * dinv[dst], dinv * dinv]).astype(np.float32)

    blk = edst >> 7          # dst block
    dnl = (edst & 127).astype(np.float32)
    n_blocks = N_CORES * bpc
    low = esrc < HALF

    # order edges by (block, highness) once
    order = np.lexsort((~low, blk))
    esrc, dnl_s, enrm_s, blk_s, low_s = (
        esrc[order], dnl[order], enrm[order], blk[order], low[order])

    # per-block counts of low/high edges, per core
    nlow = np.zeros((n_blocks,), np.int64)
    nhigh = np.zeros((n_blocks,), np.int64)
    cnts = np.bincount(blk_s * 2 + (~low_s).astype(np.int64), minlength=2 * n_blocks)
    nlow = cnts[0::2]
    nhigh = cnts[1::2]

    # uniform chunk counts per local block index (max over cores)
    nlow_2d = nlow.reshape(N_CORES, bpc)
    nhigh_2d = nhigh.reshape(N_CORES, bpc)
    cntl = np.maximum(np.ceil(nlow_2d / P).astype(np.int64).max(axis=0), 0)
    cnth = np.maximum(np.ceil(nhigh_2d / P).astype(np.int64).max(axis=0), 0)
    # every block needs at least one chunk so its PSUM group exists
    zero = (cntl + cnth) == 0
    cntl[zero] = 1

    C = int((cntl + cnth).sum())  # chunks per core
    # per-core slot arrays [C*P]
    gidx = np.zeros((N_CORES, C * P), np.int64)
    gdnl = np.zeros((N_CORES, C * P), np.float32)
    gnrm = np.zeros((N_CORES, C * P), np.float32)

    # block start offsets in the sorted edge array
    blk_starts = np.zeros(n_blocks + 1, np.int64)
    np.cumsum(nlow + nhigh, out=blk_starts[1:])

    for c in range(N_CORES):
        pos = 0
        for i in range(bpc):
            b = c * bpc + i
            s0 = blk_starts[b]
            nl, nh = nlow[b], nhigh[b]
            # low slots
            ncap = int(cntl[i]) * P
            take = min(nl, ncap)
            sl = slice(pos, pos + take)
            gidx[c, sl] = esrc[s0:s0 + take]
            gdnl[c, sl] = dnl_s[s0:s0 + take]
            gnrm[c, sl] = enrm_s[s0:s0 + take]
            pos += ncap
            # high slots (stored index is src - HALF)
            hcap = int(cnth[i]) * P
            takeh = min(nh, hcap)
            sh = slice(pos, pos + takeh)
            gidx[c, sh] = esrc[s0 + nl:s0 + nl + takeh] - HALF
            gdnl[c, sh] = dnl_s[s0 + nl:s0 + nl + takeh]
            gnrm[c, sh] = enrm_s[s0 + nl:s0 + nl + takeh]
            pos += hcap
        assert pos == C * P

    # idx stream wrapped for dma_gather: [128, C*P/16] int16
    eidx = np.stack([_wrap16(gidx[c]) for c in range(N_CORES)])
    # dnl/nrm streams indexed [slot partition, chunk]: slot i of chunk k is
    # gather position k*128+i -> array [c, k*128+i] -> reshape [C,P].T
    ednl = np.ascontiguousarray(gdnl.reshape(N_CORES, C, P).transpose(0, 2, 1))
    enrmt = np.ascontiguousarray(gnrm.reshape(N_CORES, C, P).transpose(0, 2, 1))
    return dict(eidx=eidx, ednl=ednl, enrm=enrmt,
                cntl=cntl.astype(int), cnth=cnth.astype(int), C=C)


def prepare_labels(edge_label_index, n_label):
    """Bucket labels by (a<HALF, b<HALF) per core, pad to 128 multiples.

    Returns per-core idx streams for a and b sides, bucket chunk counts
    (uniform across cores), and the per-core slot->label mapping.
    """
    a = np.asarray(edge_label_index[0], dtype=np.int64)
    b = np.asarray(edge_label_index[1], dtype=np.int64)
    per = n_label // N_CORES
    buckets_per_core = []
    for c in range(N_CORES):
        la = a[c * per:(c + 1) * per]
        lb = b[c * per:(c + 1) * per]
        lab = np.arange(c * per, (c + 1) * per)
        bid = (la >= HALF) * 2 + (lb >= HALF)
        buckets_per_core.append([(la[bid == k], lb[bid == k], lab[bid == k])
                                 for k in range(4)])
    tcnt = [max(int(np.ceil(len(buckets_per_core[c][k][0]) / P))
                for c in range(N_CORES)) for k in range(4)]
    T = sum(tcnt)
    aidx = np.zeros((N_CORES, T * P), np.int64)
    bidx = np.zeros((N_CORES, T * P), np.int64)
    labmap = np.full((N_CORES, T * P), -1, np.int64)
    for c in range(N_CORES):
        pos = 0
        for k in range(4):
            la, lb, lab = buckets_per_core[c][k]
            n = len(la)
            cap = tcnt[k] * P
            aidx[c, pos:pos + n] = la - (HALF if k >= 2 else 0)
            bidx[c, pos:pos + n] = lb - (HALF if k % 2 else 0)
            labmap[c, pos:pos + n] = lab
            pos += cap
    la_s = np.stack([_wrap16(aidx[c]) for c in range(N_CORES)])
    lb_s = np.stack([_wrap16(bidx[c]) for c in range(N_CORES)])
    return dict(la=la_s, lb=lb_s, tcnt=tcnt, T=T, labmap=labmap)


# ------------------------------------------------------------- device kernel

def build_bass(n_nodes, bpc, cntl, cnth, tcnt, in_c, hid_c, out_c):
    from concourse import bacc, bass, mybir
    import concourse.tile as tile

    NPAD = N_CORES * bpc * P
    C = int(sum(cntl) + sum(cnth))
    T = int(sum(tcnt))
    f32 = mybir.dt.float32

    nc = bacc.Bacc("TRN2", target_bir_lowering=False, debug=False,
                   num_devices=N_CORES, num_swdge_queues=4)

    x_d = nc.dram_tensor("x", [n_nodes, in_c], f32, kind="ExternalInput")
    w_d = [nc.dram_tensor(f"W{i+1}", s, f32, kind="ExternalInput")
           for i, s in enumerate([[in_c, hid_c], [hid_c, hid_c], [hid_c, out_c]])]
    b_d = [nc.dram_tensor(f"b{i+1}", [s], f32, kind="ExternalInput")
           for i, s in enumerate([hid_c, hid_c, out_c])]
    eidx_d = nc.dram_tensor("eidx", [P, C * P // 16], mybir.dt.int16,
                            kind="ExternalInput")
    ednl_d = nc.dram_tensor("ednl", [P, C], f32, kind="ExternalInput")
    enrm_d = nc.dram_tensor("enrm", [P, C], f32, kind="ExternalInput")
    la_d = nc.dram_tensor("la", [P, T * P // 16], mybir.dt.int16,
                          kind="ExternalInput")
    lb_d = nc.dram_tensor("lb", [P, T * P // 16], mybir.dt.int16,
                          kind="ExternalInput")
    out_d = nc.dram_tensor("out", [P, T], f32, kind="ExternalOutput")

    # internal DRAM: per-layer z slice (local) + allgathered z (shared)
    zs_d = [nc.dram_tensor(f"zs{l}", [bpc * P, w], f32, kind="Internal")
            for l, w in enumerate([hid_c, hid_c, out_c])]
    zf_d = [nc.dram_tensor(f"zf{l}", [NPAD, w], f32, kind="Internal",
                           addr_space="Shared")
            for l, w in enumerate([hid_c, hid_c, out_c])]

    gq = [0]  # round-robin swdge queue

    def next_q():
        q = gq[0]
        gq[0] = (q + 1) % 4
        return q

    with tile.TileContext(nc) as tc:
        with (
            tc.tile_pool(name="consts", bufs=1) as cst,
            tc.tile_pool(name="gath", bufs=6) as gp,
            tc.tile_pool(name="dec", bufs=1) as dp,
            tc.tile_pool(name="work", bufs=8) as wp,
            tc.tile_pool(name="outp", bufs=4) as op,
            tc.tile_pool(name="psum", bufs=4, space="PSUM") as ps,
        ):
            # ---- constants and streams (resident whole kernel)
            iota = cst.tile([P, P], f32)
            nc.gpsimd.iota(iota[:], pattern=[[1, P]], base=0,
                           channel_multiplier=0,
                           allow_small_or_imprecise_dtypes=True)
            ones1 = cst.tile([1, P], f32)
            nc.vector.memset(ones1[:], 1.0)

            eidx_sb = cst.tile([P, C * P // 16], mybir.dt.int16)
            ednl_sb = cst.tile([P, C], f32)
            enrm_sb = cst.tile([P, C], f32)
            nc.sync.dma_start(eidx_sb[:], eidx_d[:, :])
            nc.sync.dma_start(ednl_sb[:], ednl_d[:, :])
            nc.sync.dma_start(enrm_sb[:], enrm_d[:, :])
            la_sb = cst.tile([P, T * P // 16], mybir.dt.int16)
            lb_sb = cst.tile([P, T * P // 16], mybir.dt.int16)
            nc.sync.dma_start(la_sb[:], la_d[:, :])
            nc.sync.dma_start(lb_sb[:], lb_d[:, :])

            w_sb = []
            bias_sb = []
            for l in range(3):
                wt = cst.tile([hid_c if l else in_c, out_c if l == 2 else hid_c], f32)
                nc.sync.dma_start(wt[:], w_d[l][:, :])
                w_sb.append(wt)
                bt = cst.tile([1, out_c if l == 2 else hid_c], f32)
                nc.sync.dma_start(bt[:], b_d[l][None, :])
                bias_sb.append(bt)

            # ---- 3 GCN layers
            for l in range(3):
                oc = out_c if l == 2 else hid_c
                if l == 0:
                    lo_tab = x_d[:, :]
                    hi_tab = x_d[HALF:, :] if n_nodes > HALF else None
                else:
                    zprev = zf_d[l - 1]
                    lo_tab = zprev[:, :]
                    hi_tab = zprev[HALF:, :] if NPAD > HALF else None

                chunk_base = 0
                for i in range(bpc):
                    cl, ch = int(cntl[i]), int(cnth[i])
                    cnt = cl + ch
                    gt = gp.tile([P, cnt * in_c], f32, tag="gath")
                    g3 = gt[:].rearrange("p (c f) -> p c f", c=cnt)
                    if cl:
                        nc.gpsimd.dma_gather(
                            out_ap=g3[:, 0:cl, :] if ch else g3,
                            in_ap=lo_tab,
                            idxs_ap=eidx_sb[:, chunk_base * 8:(chunk_base + cl) * 8],
                            num_idxs=cl * P, num_idxs_reg=cl * P,
                            elem_size=in_c,
                            single_packet=False, queue_num=next_q())
                    if ch:
                        nc.gpsimd.dma_gather(
                            out_ap=g3[:, cl:, :] if cl else g3,
                            in_ap=hi_tab,
                            idxs_ap=eidx_sb[:, (chunk_base + cl) * 8:
                                            (chunk_base + cnt) * 8],
                            num_idxs=ch * P, num_idxs_reg=ch * P,
                            elem_size=in_c,
                            single_packet=False, queue_num=next_q())

                    agg_ps = ps.tile([P, P], f32, tag="agg", space="PSUM")
                    for k in range(cnt):
                        ind = wp.tile([P, P], f32, tag="ind")
                        nc.vector.tensor_scalar(
                            out=ind[:], in0=iota[:],
                            scalar1=ednl_sb[:, chunk_base + k:chunk_base + k + 1],
                            scalar2=enrm_sb[:, chunk_base + k:chunk_base + k + 1],
                            op0=mybir.AluOpType.is_equal,
                            op1=mybir.AluOpType.mult)
                        nc.tensor.matmul(
                            out=agg_ps[:], lhsT=g3[:, k, :], rhs=ind[:],
                            start=(k == 0), stop=(k == cnt - 1))

                    aggT = wp.tile([P, P], f32, tag="aggT")
                    nc.vector.tensor_copy(out=aggT[:], in_=agg_ps[:])

                    z_ps = ps.tile([P, oc], f32, tag="z", space="PSUM")
                    nc.tensor.matmul(out=z_ps[:], lhsT=ones1[:],
                                     rhs=bias_sb[l][:], start=True, stop=False)
                    nc.tensor.matmul(out=z_ps[:], lhsT=aggT[:], rhs=w_sb[l][:],
                                     start=False, stop=True)

                    z_sb = op.tile([P, oc], f32, tag="z_sb")
                    if l < 2:
                        nc.vector.tensor_scalar_max(out=z_sb[:], in0=z_ps[:],
                                                    scalar1=0.0)
                    else:
                        nc.vector.tensor_copy(out=z_sb[:], in_=z_ps[:])
                    nc.sync.dma_start(zs_d[l][i * P:(i + 1) * P, :], z_sb[:])
                    chunk_base += cnt

                nc.gpsimd.collective_compute(
                    "AllGather", mybir.AluOpType.bypass,
                    replica_groups=[list(range(N_CORES))],
                    ins=[zs_d[l][:, :]], outs=[zf_d[l][:, :]])

            # ---- decode
            z3 = zf_d[2]
            tbase = 0
            res = cst.tile([P, T], f32)
            for k in range(4):
                tk = int(tcnt[k])
                if tk == 0:
                    continue
                a_tab = z3[HALF:, :] if (k >= 2 and NPAD > HALF) else z3[:, :]
                b_tab = z3[HALF:, :] if (k % 2 and NPAD > HALF) else z3[:, :]
                ga = dp.tile([P, tk * out_c], f32, tag="ga")
                gb = dp.tile([P, tk * out_c], f32, tag="gb")
                nc.gpsimd.dma_gather(
                    out_ap=ga[:].rearrange("p (c f) -> p c f", c=tk),
                    in_ap=a_tab,
                    idxs_ap=la_sb[:, tbase * 8:(tbase + tk) * 8],
                    num_idxs=tk * P, num_idxs_reg=tk * P, elem_size=out_c,
                    single_packet=False, queue_num=next_q())
                nc.gpsimd.dma_gather(
                    out_ap=gb[:].rearrange("p (c f) -> p c f", c=tk),
                    in_ap=b_tab,
                    idxs_ap=lb_sb[:, tbase * 8:(tbase + tk) * 8],
                    num_idxs=tk * P, num_idxs_reg=tk * P, elem_size=out_c,
                    single_packet=False, queue_num=next_q())
                nc.vector.tensor_mul(out=ga[:], in0=ga[:], in1=gb[:])
                nc.vector.tensor_reduce(
                    out=res[:, tbase:tbase + tk],
                    in_=ga[:].rearrange("p (c f) -> p c f", c=tk),
                    axis=mybir.AxisListType.X, op=mybir.AluOpType.add)
                tbase += tk
            nc.sync.dma_start(out_d[:, :], res[:])

    nc.finalize()
    return nc


# ---------------------------------------------------------------- entry point

def kernel(x, W1, b1, W2, b2, W3, b3, edge_index, edge_label_index):
    from concourse.bass_utils import run_bass_kernel_spmd

    x = np.ascontiguousarray(np.asarray(x, dtype=np.float32))
    n_nodes, in_c = x.shape
    hid_c = np.asarray(W2).shape[0]
    out_c = np.asarray(W3).shape[1]
    n_label = np.asarray(edge_label_index).shape[1]
    bpc = int(np.ceil(n_nodes / (N_CORES * P)))

    ed = prepare_edges(edge_index, n_nodes, bpc)
    lb = prepare_labels(edge_label_index, n_label)

    nc = build_bass(n_nodes, bpc, ed["cntl"], ed["cnth"], lb["tcnt"],
                    in_c, hid_c, out_c)

    common = {
        "x": x,
        "W1": np.ascontiguousarray(np.asarray(W1, np.float32)),
        "W2": np.ascontiguousarray(np.asarray(W2, np.float32)),
        "W3": np.ascontiguousarray(np.asarray(W3, np.float32)),
        "b1": np.ascontiguousarray(np.asarray(b1, np.float32)),
        "b2": np.ascontiguousarray(np.asarray(b2, np.float32)),
        "b3": np.ascontiguousarray(np.asarray(b3, np.float32)),
    }
    in_maps = []
    for c in range(N_CORES):
        m = dict(common)
        m["eidx"] = np.ascontiguousarray(ed["eidx"][c])
        m["ednl"] = np.ascontiguousarray(ed["ednl"][c])
        m["enrm"] = np.ascontiguousarray(ed["enrm"][c])
        m["la"] = np.ascontiguousarray(lb["la"][c])
        m["lb"] = np.ascontiguousarray(lb["lb"][c])
        in_maps.append(m)

    res = run_bass_kernel_spmd(nc, in_maps, core_ids=list(range(N_CORES)))

    out = np.zeros((n_label,), np.float32)
    for c in range(N_CORES):
        o = res.results[c]["out"]  # [P, T]
        flat = o.T.reshape(-1)  # slot i at [i%128, i//128] -> o.T.flat[i]
        lm = lb["labmap"][c]
        valid = lm >= 0
        out[lm[valid]] = flat[valid]
    return out



# revision 3
# speedup vs baseline: 1.4580x; 1.4580x over previous
"""GCN message-passing kernel for Trainium2, 8 NeuronCores (v3).

Math (reference): 3-layer GCN with symmetric normalization and self-loops,
then dot-product decode over label edge pairs.

Reformulation: A_hat @ (x @ W) == (A_hat @ x) @ W, so each layer is
  agg = A_hat @ z          (sparse gather + PE indicator-matmul scatter)
  z   = relu(agg @ W + b)
A_hat is shared by all 3 layers; normalization folded into per-edge values.

v3 design:
- fp16 data path (tables/weights/messages/indicators); z3 + decode fp32.
- dst blocks assigned to cores snake-wise by edge count so the SPMD
  program's per-block chunk counts (max over cores) have ~zero padding.
- Scatter indicators PRECOMPUTED host-side (fp16, one [128,128] tile per
  128-edge chunk) and streamed from DRAM via HWDGE dma_start per group —
  zero per-chunk DVE work (a DVE op has a ~270ns fixed cost, too slow for
  ~850 chunks x 3 layers).
- Self-loops via one contiguous HWDGE dma_start from the core's own
  previous-layer slice plus a resident diag(dinv^2) indicator
  (start=True clears psum for the block).
- Edge gathers merged per group of G=7 blocks (2 SWDGE calls per group)
  to amortize the ~1us/call Q7 emission cost; GpSimd runs ONLY gather
  emission.
- psum->sbuf agg copy on Scalar (closer to PSUM); relu+cast on Vector.
"""

import numpy as np

P = 128
HALF = 32768
N_CORES = 8
BPC = 49              # blocks per core (392 / 8)
G = 7                 # blocks per gather group
N_NODES_PAD = N_CORES * BPC * P


# ---------------------------------------------------------------- host prep

def _wrap16(flat_idx):
    """dma_gather idx layout: idx i at [i%16, i//16], replicated to 128 rows."""
    t = flat_idx.astype(np.int16).reshape(-1, 16).T  # [16, n/16]
    return np.tile(t, (8, 1))  # [128, n/16]


def prepare(edge_index, n_nodes):
    """Snake-balance blocks, build per-core chunk streams + indicators."""
    src = np.asarray(edge_index[0], dtype=np.int64)
    dst = np.asarray(edge_index[1], dtype=np.int64)
    deg = np.bincount(dst, minlength=N_NODES_PAD).astype(np.float64) + 1.0
    dinv = (1.0 / np.sqrt(deg)).astype(np.float32)

    n_blocks = N_CORES * BPC
    blk_of_dst = dst >> 7
    blk_edges = np.bincount(blk_of_dst, minlength=n_blocks)

    order = np.argsort(-blk_edges, kind="stable")
    blk_core = np.empty(n_blocks, np.int64)
    blk_local = np.empty(n_blocks, np.int64)
    for r, b in enumerate(order):
        rnd, pos = divmod(r, N_CORES)
        core = pos if rnd % 2 == 0 else N_CORES - 1 - pos
        blk_core[b] = core
        blk_local[b] = rnd
    zfrow_blk = blk_core * BPC * P + blk_local * P
    zfrow = (zfrow_blk[np.arange(N_NODES_PAD) >> 7]
             + (np.arange(N_NODES_PAD) & 127))

    srow = zfrow[src]
    drow = zfrow[dst]
    enrm = (dinv[src] * dinv[dst]).astype(np.float32)

    dblk = drow >> 7
    dnl = (drow & 127).astype(np.int32)
    high = srow >= HALF

    so = np.lexsort((dnl, high, dblk))
    srow_s, dnl_s, enrm_s, high_s = srow[so], dnl[so], enrm[so], high[so]
    dblk_s = dblk[so]

    cnts = np.bincount(dblk_s * 2 + high_s, minlength=2 * n_blocks)
    nlow = cnts[0::2].reshape(N_CORES, BPC)
    nhigh = cnts[1::2].reshape(N_CORES, BPC)
    cntl = np.ceil(nlow / P).astype(np.int64).max(axis=0)
    cnth = np.ceil(nhigh / P).astype(np.int64).max(axis=0)

    C = int((cntl + cnth).sum())
    gidx = np.zeros((N_CORES, C * P), np.int64)
    gdnl = np.zeros((N_CORES, C * P), np.int32)
    gnrm = np.zeros((N_CORES, C * P), np.float32)

    blk_starts = np.zeros(2 * n_blocks + 1, np.int64)
    np.cumsum(cnts, out=blk_starts[1:])

    n_groups = BPC // G
    posl = np.zeros(BPC, np.int64)
    posh = np.zeros(BPC, np.int64)
    pos = 0
    grp_cntl = np.zeros(n_groups, np.int64)
    grp_cnth = np.zeros(n_groups, np.int64)
    for g in range(n_groups):
        for i in range(g * G, (g + 1) * G):
            posl[i] = pos
            pos += cntl[i]
        for i in range(g * G, (g + 1) * G):
            posh[i] = pos
            pos += cnth[i]
        grp_cntl[g] = cntl[g * G:(g + 1) * G].sum()
        grp_cnth[g] = cnth[g * G:(g + 1) * G].sum()
    assert pos == C

    for c in range(N_CORES):
        for i in range(BPC):
            b = c * BPC + i
            s0, n = blk_starts[2 * b], nlow.reshape(-1)[b]
            sl = slice(posl[i] * P, posl[i] * P + n)
            gidx[c, sl] = srow_s[s0:s0 + n]
            gdnl[c, sl] = dnl_s[s0:s0 + n]
            gnrm[c, sl] = enrm_s[s0:s0 + n]
            s0, n = blk_starts[2 * b + 1], nhigh.reshape(-1)[b]
            sh = slice(posh[i] * P, posh[i] * P + n)
            gidx[c, sh] = srow_s[s0:s0 + n] - HALF
            gdnl[c, sh] = dnl_s[s0:s0 + n]
            gnrm[c, sh] = enrm_s[s0:s0 + n]

    eidx = np.stack([_wrap16(gidx[c]) for c in range(N_CORES)])

    # edge-chunk indicators: [core][slot=128, C*128] fp16;
    # chunk ck column block holds ind[slot, d] = nrm iff dnl==d
    ind_edge = np.zeros((N_CORES, P, C * P), np.float16)
    allpos = np.arange(C * P)
    slot = allpos % P
    ck = allpos // P
    for c in range(N_CORES):
        nz = gnrm[c] != 0
        ind_edge[c, slot[nz], ck[nz] * P + gdnl[c, nz]] = \
            gnrm[c, nz].astype(np.float16)

    # self-loop diag indicators
    dinv2 = (dinv * dinv).astype(np.float32)
    node_of_row = np.empty(N_NODES_PAD, np.int64)
    node_of_row[zfrow] = np.arange(N_NODES_PAD)
    dinv2_row = dinv2[node_of_row]
    ind_self = np.zeros((N_CORES, P, BPC * P), np.float16)
    ar = np.arange(P)
    for c in range(N_CORES):
        for i in range(BPC):
            rows = dinv2_row[(c * BPC + i) * P:(c * BPC + i + 1) * P]
            ind_self[c, ar, i * P + ar] = rows.astype(np.float16)

    layout = dict(cntl=cntl.astype(int), cnth=cnth.astype(int),
                  posl=posl, posh=posh, grp_cntl=grp_cntl.astype(int),
                  grp_cnth=grp_cnth.astype(int), C=C, n_groups=n_groups,
                  zfrow=zfrow)
    data = dict(eidx=eidx, ind_edge=ind_edge, ind_self=ind_self)
    return layout, data


def prepare_labels(edge_label_index, n_label, zfrow):
    """Bucket labels by (a<HALF, b<HALF) per core, pad to 128 multiples."""
    a = zfrow[np.asarray(edge_label_index[0], dtype=np.int64)]
    b = zfrow[np.asarray(edge_label_index[1], dtype=np.int64)]
    per = n_label // N_CORES
    buckets = []
    for c in range(N_CORES):
        la = a[c * per:(c + 1) * per]
        lb = b[c * per:(c + 1) * per]
        lab = np.arange(c * per, (c + 1) * per)
        bid = (la >= HALF) * 2 + (lb >= HALF)
        buckets.append([(la[bid == k], lb[bid == k], lab[bid == k])
                        for k in range(4)])
    tcnt = [max(int(np.ceil(len(buckets[c][k][0]) / P))
                for c in range(N_CORES)) for k in range(4)]
    T = sum(tcnt)
    aidx = np.zeros((N_CORES, T * P), np.int64)
    bidx = np.zeros((N_CORES, T * P), np.int64)
    labmap = np.full((N_CORES, T * P), -1, np.int64)
    for c in range(N_CORES):
        pos = 0
        for k in range(4):
            la, lb, lab = buckets[c][k]
            n = len(la)
            aidx[c, pos:pos + n] = la - (HALF if k >= 2 else 0)
            bidx[c, pos:pos + n] = lb - (HALF if k % 2 else 0)
            labmap[c, pos:pos + n] = lab
            pos += tcnt[k] * P
    la_s = np.stack([_wrap16(aidx[c]) for c in range(N_CORES)])
    lb_s = np.stack([_wrap16(bidx[c]) for c in range(N_CORES)])
    return dict(la=la_s, lb=lb_s, tcnt=tcnt, T=T, labmap=labmap)


# ------------------------------------------------------------- device kernel

def build_bass(lay, tcnt, in_c, hid_c, out_c):
    from concourse import bacc, bass, mybir
    import concourse.tile as tile

    C = lay["C"]
    T = int(sum(tcnt))
    n_groups = lay["n_groups"]
    cntl, cnth = lay["cntl"], lay["cnth"]
    posl, posh = lay["posl"], lay["posh"]
    grp_cntl, grp_cnth = lay["grp_cntl"], lay["grp_cnth"]
    f32 = mybir.dt.float32
    f16 = mybir.dt.float16
    i16 = mybir.dt.int16

    nc = bacc.Bacc("TRN2", target_bir_lowering=False, debug=False,
                   num_devices=N_CORES, num_swdge_queues=4)

    xf_d = nc.dram_tensor("xf", [N_NODES_PAD, in_c], f16, kind="ExternalInput")
    xs_d = nc.dram_tensor("xs", [BPC * P, in_c], f16, kind="ExternalInput")
    w_d = [nc.dram_tensor(f"W{i+1}", s, f16, kind="ExternalInput")
           for i, s in enumerate([[in_c, hid_c], [hid_c, hid_c],
                                  [hid_c, out_c]])]
    b_d = [nc.dram_tensor(f"b{i+1}", [s], f16, kind="ExternalInput")
           for i, s in enumerate([hid_c, hid_c, out_c])]
    eidx_d = nc.dram_tensor("eidx", [P, C * 8], i16, kind="ExternalInput")
    inde_d = nc.dram_tensor("inde", [P, C * P], f16, kind="ExternalInput")
    inds_d = nc.dram_tensor("inds", [P, BPC * P], f16, kind="ExternalInput")
    la_d = nc.dram_tensor("la", [P, T * 8], i16, kind="ExternalInput")
    lb_d = nc.dram_tensor("lb", [P, T * 8], i16, kind="ExternalInput")
    out_d = nc.dram_tensor("out", [P, T], f32, kind="ExternalOutput")

    zs_d = [nc.dram_tensor(f"zs{l}", [BPC * P, w], dt, kind="Internal")
            for l, (w, dt) in enumerate([(hid_c, f16), (hid_c, f16),
                                         (out_c, f32)])]
    zf_d = [nc.dram_tensor(f"zf{l}", [N_NODES_PAD, w], dt, kind="Internal",
                           addr_space="Shared")
            for l, (w, dt) in enumerate([(hid_c, f16), (hid_c, f16),
                                         (out_c, f32)])]

    gq = [0]

    def next_q():
        q = gq[0]
        gq[0] = (q + 1) % 4
        return q

    with tile.TileContext(nc) as tc:
        with (
            tc.tile_pool(name="consts", bufs=1) as cst,
            tc.tile_pool(name="gath", bufs=2) as gp,
            tc.tile_pool(name="indp", bufs=2) as ip,
            tc.tile_pool(name="selfg", bufs=4) as sp,
            tc.tile_pool(name="outp", bufs=4) as op,
            tc.tile_pool(name="dec", bufs=2) as dp,
            tc.tile_pool(name="psA", bufs=4, space="PSUM") as psA,
            tc.tile_pool(name="psZ", bufs=2, space="PSUM") as psZ,
        ):
            # ---- resident constants
            ones1 = cst.tile([1, P], f16)
            nc.vector.memset(ones1[:], 1.0)

            eidx_sb = cst.tile([P, C * 8], i16)
            nc.sync.dma_start(eidx_sb[:], eidx_d[:, :])
            inds_sb = cst.tile([P, BPC * P], f16)
            nc.sync.dma_start(inds_sb[:], inds_d[:, :])
            la_sb = cst.tile([P, T * 8], i16)
            nc.sync.dma_start(la_sb[:], la_d[:, :])
            lb_sb = cst.tile([P, T * 8], i16)
            nc.sync.dma_start(lb_sb[:], lb_d[:, :])

            w_sb, bias_sb = [], []
            for l in range(3):
                wt = cst.tile([hid_c if l else in_c,
                               out_c if l == 2 else hid_c], f16)
                nc.sync.dma_start(wt[:], w_d[l][:, :])
                w_sb.append(wt)
                bt = cst.tile([1, out_c if l == 2 else hid_c], f16)
                nc.sync.dma_start(bt[:], b_d[l][None, :])
                bias_sb.append(bt)

            # ---- 3 GCN layers
            for l in range(3):
                oc = out_c if l == 2 else hid_c
                ztype = f32 if l == 2 else f16
                lo_tab = xf_d[:, :] if l == 0 else zf_d[l - 1][:, :]
                hi_tab = (xf_d[HALF:, :] if l == 0
                          else zf_d[l - 1][HALF:, :])
                prev = xs_d if l == 0 else zs_d[l - 1]

                for g in range(n_groups):
                    gcl, gch = int(grp_cntl[g]), int(grp_cnth[g])
                    gcnt = gcl + gch
                    gbase = int(posl[g * G])
                    gt = gp.tile([P, gcnt * in_c], f16, tag="gath")
                    g3 = gt[:].rearrange("p (c f) -> p c f", c=gcnt)
                    nc.gpsimd.dma_gather(
                        out_ap=g3[:, 0:gcl, :] if gch else g3,
                        in_ap=lo_tab,
                        idxs_ap=eidx_sb[:, gbase * 8:(gbase + gcl) * 8],
                        num_idxs=gcl * P, num_idxs_reg=gcl * P,
                        elem_size=in_c, single_packet=False,
                        queue_num=next_q())
                    if gch:
                        nc.gpsimd.dma_gather(
                            out_ap=g3[:, gcl:, :],
                            in_ap=hi_tab,
                            idxs_ap=eidx_sb[:, (gbase + gcl) * 8:
                                            (gbase + gcnt) * 8],
                            num_idxs=gch * P, num_idxs_reg=gch * P,
                            elem_size=in_c, single_packet=False,
                            queue_num=next_q())

                    # streamed indicators for this group's chunks
                    it = ip.tile([P, gcnt * P], f16, tag="ind")
                    nc.sync.dma_start(
                        it[:], inde_d[:, gbase * P:(gbase + gcnt) * P])

                    for i in range(g * G, (g + 1) * G):
                        cl, ch = int(cntl[i]), int(cnth[i])
                        selfg = sp.tile([P, in_c], f16, tag="selfg")
                        nc.sync.dma_start(selfg[:],
                                          prev[i * P:(i + 1) * P, :])
                        agg_ps = psA.tile([P, P], f32, tag="agg",
                                          space="PSUM")
                        nc.tensor.matmul(
                            out=agg_ps[:], lhsT=selfg[:],
                            rhs=inds_sb[:, i * P:(i + 1) * P],
                            start=True, stop=(cl + ch == 0))
                        for which, cnt, pos0 in ((0, cl, int(posl[i])),
                                                 (1, ch, int(posh[i]))):
                            for k in range(cnt):
                                ck = pos0 + k
                                gk = ck - gbase
                                last = (which == 1 or ch == 0) and \
                                       (k == cnt - 1)
                                nc.tensor.matmul(
                                    out=agg_ps[:], lhsT=g3[:, gk, :],
                                    rhs=it[:, gk * P:(gk + 1) * P],
                                    start=False, stop=last)

                        aggT = op.tile([P, P], f16, tag="aggT")
                        nc.scalar.copy(out=aggT[:], in_=agg_ps[:])

                        z_ps = psZ.tile([P, oc], f32, tag="z", space="PSUM")
                        nc.tensor.matmul(out=z_ps[:], lhsT=ones1[:],
                                         rhs=bias_sb[l][:],
                                         start=True, stop=False)
                        nc.tensor.matmul(out=z_ps[:], lhsT=aggT[:],
                                         rhs=w_sb[l][:],
                                         start=False, stop=True)

                        z_sb = op.tile([P, oc], ztype, tag="z_sb")
                        if l < 2:
                            nc.vector.tensor_scalar_max(
                                out=z_sb[:], in0=z_ps[:], scalar1=0.0)
                        else:
                            nc.vector.tensor_copy(out=z_sb[:], in_=z_ps[:])
                        nc.sync.dma_start(zs_d[l][i * P:(i + 1) * P, :],
                                          z_sb[:])

                nc.gpsimd.collective_compute(
                    "AllGather", mybir.AluOpType.bypass,
                    replica_groups=[list(range(N_CORES))],
                    ins=[zs_d[l][:, :]], outs=[zf_d[l][:, :]])

            # ---- decode (pieces of 32 chunks to bound SBUF)
            z3 = zf_d[2]
            res = cst.tile([P, T], f32)
            tbase = 0
            for k in range(4):
                tk = int(tcnt[k])
                if tk == 0:
                    continue
                a_tab = z3[HALF:, :] if k >= 2 else z3[:, :]
                b_tab = z3[HALF:, :] if k % 2 else z3[:, :]
                for h0 in range(0, tk, 32):
                    hk = min(32, tk - h0)
                    ga = dp.tile([P, 32 * out_c], f32, tag="ga")
                    gb = dp.tile([P, 32 * out_c], f32, tag="gb")
                    ga3 = ga[:, :hk * out_c].rearrange(
                        "p (c f) -> p c f", c=hk)
                    gb3 = gb[:, :hk * out_c].rearrange(
                        "p (c f) -> p c f", c=hk)
                    t0 = tbase + h0
                    nc.gpsimd.dma_gather(
                        out_ap=ga3, in_ap=a_tab,
                        idxs_ap=la_sb[:, t0 * 8:(t0 + hk) * 8],
                        num_idxs=hk * P, num_idxs_reg=hk * P,
                        elem_size=out_c, single_packet=False,
                        queue_num=next_q())
                    nc.gpsimd.dma_gather(
                        out_ap=gb3, in_ap=b_tab,
                        idxs_ap=lb_sb[:, t0 * 8:(t0 + hk) * 8],
                        num_idxs=hk * P, num_idxs_reg=hk * P,
                        elem_size=out_c, single_packet=False,
                        queue_num=next_q())
                    nc.vector.tensor_mul(out=ga[:, :hk * out_c],
                                         in0=ga[:, :hk * out_c],
                                         in1=gb[:, :hk * out_c])
                    nc.vector.tensor_reduce(
                        out=res[:, t0:t0 + hk], in_=ga3,
                        axis=mybir.AxisListType.X, op=mybir.AluOpType.add)
                tbase += tk
            nc.sync.dma_start(out_d[:, :], res[:])

    nc.finalize()
    return nc


# ---------------------------------------------------------------- entry point

def kernel(x, W1, b1, W2, b2, W3, b3, edge_index, edge_label_index):
    from concourse.bass_utils import run_bass_kernel_spmd

    x = np.asarray(x, dtype=np.float32)
    n_nodes, in_c = x.shape
    hid_c = np.asarray(W2).shape[0]
    out_c = np.asarray(W3).shape[1]
    n_label = np.asarray(edge_label_index).shape[1]

    lay, data = prepare(edge_index, n_nodes)
    lb = prepare_labels(edge_label_index, n_label, lay["zfrow"])

    nc = build_bass(lay, lb["tcnt"], in_c, hid_c, out_c)

    xf = np.zeros((N_NODES_PAD, in_c), np.float16)
    xf[lay["zfrow"][:n_nodes]] = x[:n_nodes].astype(np.float16)

    common = {
        "xf": xf,
        "W1": np.asarray(W1).astype(np.float16),
        "W2": np.asarray(W2).astype(np.float16),
        "W3": np.asarray(W3).astype(np.float16),
        "b1": np.asarray(b1).astype(np.float16),
        "b2": np.asarray(b2).astype(np.float16),
        "b3": np.asarray(b3).astype(np.float16),
    }
    in_maps = []
    for c in range(N_CORES):
        m = dict(common)
        m["xs"] = np.ascontiguousarray(
            xf[c * BPC * P:(c + 1) * BPC * P])
        m["eidx"] = np.ascontiguousarray(data["eidx"][c])
        m["inde"] = np.ascontiguousarray(data["ind_edge"][c])
        m["inds"] = np.ascontiguousarray(data["ind_self"][c])
        m["la"] = np.ascontiguousarray(lb["la"][c])
        m["lb"] = np.ascontiguousarray(lb["lb"][c])
        in_maps.append(m)

    res = run_bass_kernel_spmd(nc, in_maps, core_ids=list(range(N_CORES)))

    out = np.zeros((n_label,), np.float32)
    for c in range(N_CORES):
        o = res.results[c]["out"]  # [P, T]
        flat = o.T.reshape(-1)
        lm = lb["labmap"][c]
        valid = lm >= 0
        out[lm[valid]] = flat[valid]
    return out


# revision 4
# speedup vs baseline: 1.6158x; 1.1083x over previous
"""GCN message-passing kernel for Trainium2, 8 NeuronCores (v3).

Math (reference): 3-layer GCN with symmetric normalization and self-loops,
then dot-product decode over label edge pairs.

Reformulation: A_hat @ (x @ W) == (A_hat @ x) @ W, so each layer is
  agg = A_hat @ z          (sparse gather + PE indicator-matmul scatter)
  z   = relu(agg @ W + b)
A_hat is shared by all 3 layers; normalization folded into per-edge values.

v3 design:
- fp16 data path (tables/weights/messages/indicators); z3 + decode fp32.
- dst blocks assigned to cores snake-wise by edge count so the SPMD
  program's per-block chunk counts (max over cores) have ~zero padding.
- Scatter indicators PRECOMPUTED host-side (fp16, one [128,128] tile per
  128-edge chunk) and streamed from DRAM via HWDGE dma_start per group —
  zero per-chunk DVE work (a DVE op has a ~270ns fixed cost, too slow for
  ~850 chunks x 3 layers).
- Self-loops via one contiguous HWDGE dma_start from the core's own
  previous-layer slice plus a resident diag(dinv^2) indicator
  (start=True clears psum for the block).
- Edge gathers merged per group of G=7 blocks (2 SWDGE calls per group)
  to amortize the ~1us/call Q7 emission cost; GpSimd runs ONLY gather
  emission.
- psum->sbuf agg copy on Scalar (closer to PSUM); relu+cast on Vector.
"""

import numpy as np

P = 128
HALF = 32768
N_CORES = 8
BPC = 49              # blocks per core (392 / 8)
G = 7                 # blocks per gather group
N_NODES_PAD = N_CORES * BPC * P


# ---------------------------------------------------------------- host prep

def _wrap16(flat_idx):
    """dma_gather idx layout: idx i at [i%16, i//16], replicated to 128 rows."""
    t = flat_idx.astype(np.int16).reshape(-1, 16).T  # [16, n/16]
    return np.tile(t, (8, 1))  # [128, n/16]


def prepare(edge_index, n_nodes):
    """Snake-balance blocks, build per-core chunk streams + indicators."""
    src = np.asarray(edge_index[0], dtype=np.int64)
    dst = np.asarray(edge_index[1], dtype=np.int64)
    deg = np.bincount(dst, minlength=N_NODES_PAD).astype(np.float64) + 1.0
    dinv = (1.0 / np.sqrt(deg)).astype(np.float32)

    n_blocks = N_CORES * BPC
    blk_of_dst = dst >> 7
    blk_edges = np.bincount(blk_of_dst, minlength=n_blocks)

    order = np.argsort(-blk_edges, kind="stable")
    blk_core = np.empty(n_blocks, np.int64)
    blk_local = np.empty(n_blocks, np.int64)
    for r, b in enumerate(order):
        rnd, pos = divmod(r, N_CORES)
        core = pos if rnd % 2 == 0 else N_CORES - 1 - pos
        blk_core[b] = core
        blk_local[b] = rnd
    zfrow_blk = blk_core * BPC * P + blk_local * P
    zfrow = (zfrow_blk[np.arange(N_NODES_PAD) >> 7]
             + (np.arange(N_NODES_PAD) & 127))

    srow = zfrow[src]
    drow = zfrow[dst]
    enrm = (dinv[src] * dinv[dst]).astype(np.float32)

    dblk = drow >> 7
    dnl = (drow & 127).astype(np.int32)
    high = srow >= HALF

    so = np.lexsort((dnl, high, dblk))
    srow_s, dnl_s, enrm_s, high_s = srow[so], dnl[so], enrm[so], high[so]
    dblk_s = dblk[so]

    cnts = np.bincount(dblk_s * 2 + high_s, minlength=2 * n_blocks)
    nlow = cnts[0::2].reshape(N_CORES, BPC)
    nhigh = cnts[1::2].reshape(N_CORES, BPC)
    cntl = np.ceil(nlow / P).astype(np.int64).max(axis=0)
    cnth = np.ceil(nhigh / P).astype(np.int64).max(axis=0)

    C = int((cntl + cnth).sum())
    gidx = np.zeros((N_CORES, C * P), np.int64)
    gdnl = np.zeros((N_CORES, C * P), np.int32)
    gnrm = np.zeros((N_CORES, C * P), np.float32)

    blk_starts = np.zeros(2 * n_blocks + 1, np.int64)
    np.cumsum(cnts, out=blk_starts[1:])

    n_groups = BPC // G
    posl = np.zeros(BPC, np.int64)
    posh = np.zeros(BPC, np.int64)
    pos = 0
    grp_cntl = np.zeros(n_groups, np.int64)
    grp_cnth = np.zeros(n_groups, np.int64)
    for g in range(n_groups):
        for i in range(g * G, (g + 1) * G):
            posl[i] = pos
            pos += cntl[i]
        for i in range(g * G, (g + 1) * G):
            posh[i] = pos
            pos += cnth[i]
        grp_cntl[g] = cntl[g * G:(g + 1) * G].sum()
        grp_cnth[g] = cnth[g * G:(g + 1) * G].sum()
    assert pos == C

    for c in range(N_CORES):
        for i in range(BPC):
            b = c * BPC + i
            s0, n = blk_starts[2 * b], nlow.reshape(-1)[b]
            sl = slice(posl[i] * P, posl[i] * P + n)
            gidx[c, sl] = srow_s[s0:s0 + n]
            gdnl[c, sl] = dnl_s[s0:s0 + n]
            gnrm[c, sl] = enrm_s[s0:s0 + n]
            s0, n = blk_starts[2 * b + 1], nhigh.reshape(-1)[b]
            sh = slice(posh[i] * P, posh[i] * P + n)
            gidx[c, sh] = srow_s[s0:s0 + n] - HALF
            gdnl[c, sh] = dnl_s[s0:s0 + n]
            gnrm[c, sh] = enrm_s[s0:s0 + n]

    eidx = np.stack([_wrap16(gidx[c]) for c in range(N_CORES)])

    # edge-chunk indicators: [core][slot=128, C*128] fp16;
    # chunk ck column block holds ind[slot, d] = nrm iff dnl==d
    ind_edge = np.zeros((N_CORES, P, C * P), np.float16)
    allpos = np.arange(C * P)
    slot = allpos % P
    ck = allpos // P
    for c in range(N_CORES):
        nz = gnrm[c] != 0
        ind_edge[c, slot[nz], ck[nz] * P + gdnl[c, nz]] = \
            gnrm[c, nz].astype(np.float16)

    # self-loop diag indicators
    dinv2 = (dinv * dinv).astype(np.float32)
    node_of_row = np.empty(N_NODES_PAD, np.int64)
    node_of_row[zfrow] = np.arange(N_NODES_PAD)
    dinv2_row = dinv2[node_of_row]
    ind_self = np.zeros((N_CORES, P, BPC * P), np.float16)
    ar = np.arange(P)
    for c in range(N_CORES):
        for i in range(BPC):
            rows = dinv2_row[(c * BPC + i) * P:(c * BPC + i + 1) * P]
            ind_self[c, ar, i * P + ar] = rows.astype(np.float16)

    layout = dict(cntl=cntl.astype(int), cnth=cnth.astype(int),
                  posl=posl, posh=posh, grp_cntl=grp_cntl.astype(int),
                  grp_cnth=grp_cnth.astype(int), C=C, n_groups=n_groups,
                  zfrow=zfrow)
    data = dict(eidx=eidx, ind_edge=ind_edge, ind_self=ind_self)
    return layout, data


def prepare_labels(edge_label_index, n_label, zfrow):
    """Bucket labels by (a<HALF, b<HALF) per core, pad to 128 multiples."""
    a = zfrow[np.asarray(edge_label_index[0], dtype=np.int64)]
    b = zfrow[np.asarray(edge_label_index[1], dtype=np.int64)]
    per = n_label // N_CORES
    buckets = []
    for c in range(N_CORES):
        la = a[c * per:(c + 1) * per]
        lb = b[c * per:(c + 1) * per]
        lab = np.arange(c * per, (c + 1) * per)
        bid = (la >= HALF) * 2 + (lb >= HALF)
        buckets.append([(la[bid == k], lb[bid == k], lab[bid == k])
                        for k in range(4)])
    tcnt = [max(int(np.ceil(len(buckets[c][k][0]) / P))
                for c in range(N_CORES)) for k in range(4)]
    T = sum(tcnt)
    aidx = np.zeros((N_CORES, T * P), np.int64)
    bidx = np.zeros((N_CORES, T * P), np.int64)
    labmap = np.full((N_CORES, T * P), -1, np.int64)
    for c in range(N_CORES):
        pos = 0
        for k in range(4):
            la, lb, lab = buckets[c][k]
            n = len(la)
            aidx[c, pos:pos + n] = la - (HALF if k >= 2 else 0)
            bidx[c, pos:pos + n] = lb - (HALF if k % 2 else 0)
            labmap[c, pos:pos + n] = lab
            pos += tcnt[k] * P
    la_s = np.stack([_wrap16(aidx[c]) for c in range(N_CORES)])
    lb_s = np.stack([_wrap16(bidx[c]) for c in range(N_CORES)])
    return dict(la=la_s, lb=lb_s, tcnt=tcnt, T=T, labmap=labmap)


# ------------------------------------------------------------- device kernel

def build_bass(lay, tcnt, in_c, hid_c, out_c):
    from concourse import bacc, bass, mybir
    import concourse.tile as tile

    C = lay["C"]
    T = int(sum(tcnt))
    n_groups = lay["n_groups"]
    cntl, cnth = lay["cntl"], lay["cnth"]
    posl, posh = lay["posl"], lay["posh"]
    grp_cntl, grp_cnth = lay["grp_cntl"], lay["grp_cnth"]
    f32 = mybir.dt.float32
    f16 = mybir.dt.float16
    i16 = mybir.dt.int16

    nc = bacc.Bacc("TRN2", target_bir_lowering=False, debug=False,
                   num_devices=N_CORES, num_swdge_queues=4)

    xf_d = nc.dram_tensor("xf", [N_NODES_PAD, in_c], f16, kind="ExternalInput")
    xs_d = nc.dram_tensor("xs", [BPC * P, in_c], f16, kind="ExternalInput")
    w_d = [nc.dram_tensor(f"W{i+1}", s, f16, kind="ExternalInput")
           for i, s in enumerate([[in_c, hid_c], [hid_c, hid_c],
                                  [hid_c, out_c]])]
    b_d = [nc.dram_tensor(f"b{i+1}", [s], f16, kind="ExternalInput")
           for i, s in enumerate([hid_c, hid_c, out_c])]
    eidx_d = nc.dram_tensor("eidx", [P, C * 8], i16, kind="ExternalInput")
    inde_d = nc.dram_tensor("inde", [P, C * P], f16, kind="ExternalInput")
    inds_d = nc.dram_tensor("inds", [P, BPC * P], f16, kind="ExternalInput")
    la_d = nc.dram_tensor("la", [P, T * 8], i16, kind="ExternalInput")
    lb_d = nc.dram_tensor("lb", [P, T * 8], i16, kind="ExternalInput")
    out_d = nc.dram_tensor("out", [P, T], f32, kind="ExternalOutput")

    zs_d = [nc.dram_tensor(f"zs{l}", [BPC * P, w], dt, kind="Internal")
            for l, (w, dt) in enumerate([(hid_c, f16), (hid_c, f16),
                                         (out_c, f32)])]
    zf_d = [nc.dram_tensor(f"zf{l}", [N_NODES_PAD, w], dt, kind="Internal",
                           addr_space="Shared")
            for l, (w, dt) in enumerate([(hid_c, f16), (hid_c, f16),
                                         (out_c, f32)])]

    gq = [0]

    def next_q():
        q = gq[0]
        gq[0] = (q + 1) % 4
        return q

    with tile.TileContext(nc) as tc:
        with (
            tc.tile_pool(name="consts", bufs=1) as cst,
            tc.tile_pool(name="gath", bufs=2) as gp,
            tc.tile_pool(name="indp", bufs=2) as ip,
            tc.tile_pool(name="selfg", bufs=4) as sp,
            tc.tile_pool(name="outp", bufs=4) as op,
            tc.tile_pool(name="dec", bufs=2) as dp,
            tc.tile_pool(name="psA", bufs=4, space="PSUM") as psA,
            tc.tile_pool(name="psZ", bufs=2, space="PSUM") as psZ,
        ):
            # ---- resident constants
            ones1 = cst.tile([1, P], f16)
            nc.vector.memset(ones1[:], 1.0)

            eidx_sb = cst.tile([P, C * 8], i16)
            nc.sync.dma_start(eidx_sb[:], eidx_d[:, :])
            inds_sb = cst.tile([P, BPC * P], f16)
            nc.sync.dma_start(inds_sb[:], inds_d[:, :])
            la_sb = cst.tile([P, T * 8], i16)
            nc.sync.dma_start(la_sb[:], la_d[:, :])
            lb_sb = cst.tile([P, T * 8], i16)
            nc.sync.dma_start(lb_sb[:], lb_d[:, :])

            w_sb, bias_sb = [], []
            for l in range(3):
                wt = cst.tile([hid_c if l else in_c,
                               out_c if l == 2 else hid_c], f16)
                nc.sync.dma_start(wt[:], w_d[l][:, :])
                w_sb.append(wt)
                bt = cst.tile([1, out_c if l == 2 else hid_c], f16)
                nc.sync.dma_start(bt[:], b_d[l][None, :])
                bias_sb.append(bt)

            # ---- 3 GCN layers
            for l in range(3):
                oc = out_c if l == 2 else hid_c
                ztype = f32 if l == 2 else f16
                lo_tab = xf_d[:, :] if l == 0 else zf_d[l - 1][:, :]
                hi_tab = (xf_d[HALF:, :] if l == 0
                          else zf_d[l - 1][HALF:, :])
                prev = xs_d if l == 0 else zs_d[l - 1]

                for g in range(n_groups):
                    gcl, gch = int(grp_cntl[g]), int(grp_cnth[g])
                    gcnt = gcl + gch
                    gbase = int(posl[g * G])
                    gt = gp.tile([P, gcnt * in_c], f16, tag="gath")
                    g3 = gt[:].rearrange("p (c f) -> p c f", c=gcnt)
                    # split into ~4 calls on distinct queues: SWDGE
                    # descriptor emission parallelizes across queues
                    bounds = sorted({0, (gcnt + 3) // 4, gcnt // 2,
                                     3 * gcnt // 4, gcl, gcnt})
                    for a, b in zip(bounds, bounds[1:]):
                        if b <= a:
                            continue
                        tab = lo_tab if b <= gcl else hi_tab
                        nc.gpsimd.dma_gather(
                            out_ap=g3[:, a:b, :],
                            in_ap=tab,
                            idxs_ap=eidx_sb[:, (gbase + a) * 8:
                                            (gbase + b) * 8],
                            num_idxs=(b - a) * P, num_idxs_reg=(b - a) * P,
                            elem_size=in_c, single_packet=False,
                            queue_num=next_q())

                    # streamed indicators for this group's chunks
                    it = ip.tile([P, gcnt * P], f16, tag="ind")
                    nc.sync.dma_start(
                        it[:], inde_d[:, gbase * P:(gbase + gcnt) * P])

                    for i in range(g * G, (g + 1) * G):
                        cl, ch = int(cntl[i]), int(cnth[i])
                        selfg = sp.tile([P, in_c], f16, tag="selfg")
                        nc.sync.dma_start(selfg[:],
                                          prev[i * P:(i + 1) * P, :])
                        agg_ps = psA.tile([P, P], f32, tag="agg",
                                          space="PSUM")
                        nc.tensor.matmul(
                            out=agg_ps[:], lhsT=selfg[:],
                            rhs=inds_sb[:, i * P:(i + 1) * P],
                            start=True, stop=(cl + ch == 0))
                        for which, cnt, pos0 in ((0, cl, int(posl[i])),
                                                 (1, ch, int(posh[i]))):
                            for k in range(cnt):
                                ck = pos0 + k
                                gk = ck - gbase
                                last = (which == 1 or ch == 0) and \
                                       (k == cnt - 1)
                                nc.tensor.matmul(
                                    out=agg_ps[:], lhsT=g3[:, gk, :],
                                    rhs=it[:, gk * P:(gk + 1) * P],
                                    start=False, stop=last)

                        aggT = op.tile([P, P], f16, tag="aggT")
                        nc.scalar.copy(out=aggT[:], in_=agg_ps[:])

                        z_ps = psZ.tile([P, oc], f32, tag="z", space="PSUM")
                        nc.tensor.matmul(out=z_ps[:], lhsT=ones1[:],
                                         rhs=bias_sb[l][:],
                                         start=True, stop=False)
                        nc.tensor.matmul(out=z_ps[:], lhsT=aggT[:],
                                         rhs=w_sb[l][:],
                                         start=False, stop=True)

                        z_sb = op.tile([P, oc], ztype, tag="z_sb")
                        if l < 2:
                            nc.vector.tensor_scalar_max(
                                out=z_sb[:], in0=z_ps[:], scalar1=0.0)
                        else:
                            nc.vector.tensor_copy(out=z_sb[:], in_=z_ps[:])
                        nc.sync.dma_start(zs_d[l][i * P:(i + 1) * P, :],
                                          z_sb[:])

                nc.gpsimd.collective_compute(
                    "AllGather", mybir.AluOpType.bypass,
                    replica_groups=[list(range(N_CORES))],
                    ins=[zs_d[l][:, :]], outs=[zf_d[l][:, :]])

            # ---- decode (pieces of 32 chunks to bound SBUF)
            z3 = zf_d[2]
            res = cst.tile([P, T], f32)
            tbase = 0
            for k in range(4):
                tk = int(tcnt[k])
                if tk == 0:
                    continue
                a_tab = z3[HALF:, :] if k >= 2 else z3[:, :]
                b_tab = z3[HALF:, :] if k % 2 else z3[:, :]
                for h0 in range(0, tk, 32):
                    hk = min(32, tk - h0)
                    ga = dp.tile([P, 32 * out_c], f32, tag="ga")
                    gb = dp.tile([P, 32 * out_c], f32, tag="gb")
                    ga3 = ga[:, :hk * out_c].rearrange(
                        "p (c f) -> p c f", c=hk)
                    gb3 = gb[:, :hk * out_c].rearrange(
                        "p (c f) -> p c f", c=hk)
                    t0 = tbase + h0
                    nc.gpsimd.dma_gather(
                        out_ap=ga3, in_ap=a_tab,
                        idxs_ap=la_sb[:, t0 * 8:(t0 + hk) * 8],
                        num_idxs=hk * P, num_idxs_reg=hk * P,
                        elem_size=out_c, single_packet=False,
                        queue_num=next_q())
                    nc.gpsimd.dma_gather(
                        out_ap=gb3, in_ap=b_tab,
                        idxs_ap=lb_sb[:, t0 * 8:(t0 + hk) * 8],
                        num_idxs=hk * P, num_idxs_reg=hk * P,
                        elem_size=out_c, single_packet=False,
                        queue_num=next_q())
                    nc.vector.tensor_mul(out=ga[:, :hk * out_c],
                                         in0=ga[:, :hk * out_c],
                                         in1=gb[:, :hk * out_c])
                    nc.vector.tensor_reduce(
                        out=res[:, t0:t0 + hk], in_=ga3,
                        axis=mybir.AxisListType.X, op=mybir.AluOpType.add)
                tbase += tk
            nc.sync.dma_start(out_d[:, :], res[:])

    nc.finalize()
    return nc


# ---------------------------------------------------------------- entry point

def kernel(x, W1, b1, W2, b2, W3, b3, edge_index, edge_label_index):
    from concourse.bass_utils import run_bass_kernel_spmd

    x = np.asarray(x, dtype=np.float32)
    n_nodes, in_c = x.shape
    hid_c = np.asarray(W2).shape[0]
    out_c = np.asarray(W3).shape[1]
    n_label = np.asarray(edge_label_index).shape[1]

    lay, data = prepare(edge_index, n_nodes)
    lb = prepare_labels(edge_label_index, n_label, lay["zfrow"])

    nc = build_bass(lay, lb["tcnt"], in_c, hid_c, out_c)

    xf = np.zeros((N_NODES_PAD, in_c), np.float16)
    xf[lay["zfrow"][:n_nodes]] = x[:n_nodes].astype(np.float16)

    common = {
        "xf": xf,
        "W1": np.asarray(W1).astype(np.float16),
        "W2": np.asarray(W2).astype(np.float16),
        "W3": np.asarray(W3).astype(np.float16),
        "b1": np.asarray(b1).astype(np.float16),
        "b2": np.asarray(b2).astype(np.float16),
        "b3": np.asarray(b3).astype(np.float16),
    }
    in_maps = []
    for c in range(N_CORES):
        m = dict(common)
        m["xs"] = np.ascontiguousarray(
            xf[c * BPC * P:(c + 1) * BPC * P])
        m["eidx"] = np.ascontiguousarray(data["eidx"][c])
        m["inde"] = np.ascontiguousarray(data["ind_edge"][c])
        m["inds"] = np.ascontiguousarray(data["ind_self"][c])
        m["la"] = np.ascontiguousarray(lb["la"][c])
        m["lb"] = np.ascontiguousarray(lb["lb"][c])
        in_maps.append(m)

    res = run_bass_kernel_spmd(nc, in_maps, core_ids=list(range(N_CORES)))

    out = np.zeros((n_label,), np.float32)
    for c in range(N_CORES):
        o = res.results[c]["out"]  # [P, T]
        flat = o.T.reshape(-1)
        lm = lb["labmap"][c]
        valid = lm >= 0
        out[lm[valid]] = flat[valid]
    return out


# revision 8
# speedup vs baseline: 1.7852x; 1.1048x over previous
"""GCN message-passing kernel for Trainium2, 8 NeuronCores (v3).

Math (reference): 3-layer GCN with symmetric normalization and self-loops,
then dot-product decode over label edge pairs.

Reformulation: A_hat @ (x @ W) == (A_hat @ x) @ W, so each layer is
  agg = A_hat @ z          (sparse gather + PE indicator-matmul scatter)
  z   = relu(agg @ W + b)
A_hat is shared by all 3 layers; normalization folded into per-edge values.

v3 design:
- fp16 data path (tables/weights/messages/indicators); z3 + decode fp32.
- dst blocks assigned to cores snake-wise by edge count so the SPMD
  program's per-block chunk counts (max over cores) have ~zero padding.
- Scatter indicators PRECOMPUTED host-side (fp16, one [128,128] tile per
  128-edge chunk) and streamed from DRAM via HWDGE dma_start per group —
  zero per-chunk DVE work (a DVE op has a ~270ns fixed cost, too slow for
  ~850 chunks x 3 layers).
- Self-loops via one contiguous HWDGE dma_start from the core's own
  previous-layer slice plus a resident diag(dinv^2) indicator
  (start=True clears psum for the block).
- Edge gathers merged per group of G=7 blocks (2 SWDGE calls per group)
  to amortize the ~1us/call Q7 emission cost; GpSimd runs ONLY gather
  emission.
- psum->sbuf agg copy on Scalar (closer to PSUM); relu+cast on Vector.
"""

import numpy as np

P = 128
HALF = 32768
N_CORES = 8
BPC = 49              # blocks per core (392 / 8)
GROUPS = [6] * 8 + [1]   # blocks per gather group (sum = 49)
SEG0_BLOCKS = 30      # segment A = first 5 groups; AG'd while B computes
N_NODES_PAD = N_CORES * BPC * P


# ---------------------------------------------------------------- host prep

def _zf_base(core, lrow):
    """zf row of (core, local row) under the segment-major layout."""
    S0 = SEG0_BLOCKS * P
    S1 = BPC * P - S0
    if lrow < S0:
        return core * S0 + lrow
    return N_CORES * S0 + core * S1 + (lrow - S0)


def _wrap16(flat_idx):
    """dma_gather idx layout: idx i at [i%16, i//16], replicated to 128 rows."""
    t = flat_idx.astype(np.int16).reshape(-1, 16).T  # [16, n/16]
    return np.tile(t, (8, 1))  # [128, n/16]


def prepare(edge_index, n_nodes):
    """Snake-balance blocks, build per-core chunk streams + indicators."""
    src = np.asarray(edge_index[0], dtype=np.int64)
    dst = np.asarray(edge_index[1], dtype=np.int64)
    deg = np.bincount(dst, minlength=N_NODES_PAD).astype(np.float64) + 1.0
    dinv = (1.0 / np.sqrt(deg)).astype(np.float32)

    n_blocks = N_CORES * BPC
    blk_of_dst = dst >> 7
    blk_edges = np.bincount(blk_of_dst, minlength=n_blocks)

    order = np.argsort(-blk_edges, kind="stable")
    blk_core = np.empty(n_blocks, np.int64)
    blk_local = np.empty(n_blocks, np.int64)
    for r, b in enumerate(order):
        rnd, pos = divmod(r, N_CORES)
        core = pos if rnd % 2 == 0 else N_CORES - 1 - pos
        blk_core[b] = core
        blk_local[b] = rnd
    # segment-major zf layout: rows [0 : 8*S0) hold every core's first
    # S0 local rows (AG'd early), rows [8*S0 :) the rest
    S0 = SEG0_BLOCKS * P
    S1 = BPC * P - S0
    lrow = blk_local * P  # local row of block start
    seg_start = np.where(lrow < S0,
                         blk_core * S0 + lrow,
                         N_CORES * S0 + blk_core * S1 + (lrow - S0))
    zfrow = (seg_start[np.arange(N_NODES_PAD) >> 7]
             + (np.arange(N_NODES_PAD) & 127))

    srow = zfrow[src]
    drow = zfrow[dst]
    enrm = (dinv[src] * dinv[dst]).astype(np.float32)

    dblk = drow >> 7
    dnl = (drow & 127).astype(np.int32)
    high = srow >= HALF

    so = np.lexsort((dnl, high, dblk))
    srow_s, dnl_s, enrm_s, high_s = srow[so], dnl[so], enrm[so], high[so]
    dblk_s = dblk[so]

    cnts = np.bincount(dblk_s * 2 + high_s, minlength=2 * n_blocks)
    # zf block index of (core, local block) under segment-major layout
    B_of = np.empty((N_CORES, BPC), np.int64)
    for c in range(N_CORES):
        for i in range(BPC):
            B_of[c, i] = (_zf_base(c, i * P)) >> 7
    nlow = cnts[0::2][B_of]   # [core, local]
    nhigh = cnts[1::2][B_of]
    cntl = np.ceil(nlow / P).astype(np.int64).max(axis=0)
    cnth = np.ceil(nhigh / P).astype(np.int64).max(axis=0)

    C = int((cntl + cnth).sum())
    gidx = np.zeros((N_CORES, C * P), np.int64)
    gdnl = np.zeros((N_CORES, C * P), np.int32)
    gnrm = np.zeros((N_CORES, C * P), np.float32)

    blk_starts = np.zeros(2 * n_blocks + 1, np.int64)
    np.cumsum(cnts, out=blk_starts[1:])

    n_groups = len(GROUPS)
    gstart = np.cumsum([0] + GROUPS)
    posl = np.zeros(BPC, np.int64)
    posh = np.zeros(BPC, np.int64)
    pos = 0
    grp_cntl = np.zeros(n_groups, np.int64)
    grp_cnth = np.zeros(n_groups, np.int64)
    for g in range(n_groups):
        b0, b1 = int(gstart[g]), int(gstart[g + 1])
        for i in range(b0, b1):
            posl[i] = pos
            pos += cntl[i]
        for i in range(b0, b1):
            posh[i] = pos
            pos += cnth[i]
        grp_cntl[g] = cntl[b0:b1].sum()
        grp_cnth[g] = cnth[b0:b1].sum()
    assert pos == C

    for c in range(N_CORES):
        for i in range(BPC):
            b = B_of[c, i]
            s0, n = blk_starts[2 * b], nlow[c, i]
            sl = slice(posl[i] * P, posl[i] * P + n)
            gidx[c, sl] = srow_s[s0:s0 + n]
            gdnl[c, sl] = dnl_s[s0:s0 + n]
            gnrm[c, sl] = enrm_s[s0:s0 + n]
            s0, n = blk_starts[2 * b + 1], nhigh[c, i]
            sh = slice(posh[i] * P, posh[i] * P + n)
            gidx[c, sh] = srow_s[s0:s0 + n] - HALF
            gdnl[c, sh] = dnl_s[s0:s0 + n]
            gnrm[c, sh] = enrm_s[s0:s0 + n]

    eidx = np.stack([_wrap16(gidx[c]) for c in range(N_CORES)])

    # edge-chunk indicators: [core][slot=128, C*128] fp16;
    # chunk ck column block holds ind[slot, d] = nrm iff dnl==d
    ind_edge = np.zeros((N_CORES, P, C * P), np.float16)
    allpos = np.arange(C * P)
    slot = allpos % P
    ck = allpos // P
    for c in range(N_CORES):
        nz = gnrm[c] != 0
        ind_edge[c, slot[nz], ck[nz] * P + gdnl[c, nz]] = \
            gnrm[c, nz].astype(np.float16)

    # self-loop diag indicators
    dinv2 = (dinv * dinv).astype(np.float32)
    node_of_row = np.empty(N_NODES_PAD, np.int64)
    node_of_row[zfrow] = np.arange(N_NODES_PAD)
    dinv2_row = dinv2[node_of_row]
    ind_self = np.zeros((N_CORES, P, BPC * P), np.float16)
    ar = np.arange(P)
    for c in range(N_CORES):
        for i in range(BPC):
            r0 = _zf_base(c, i * P)
            rows = dinv2_row[r0:r0 + P]
            ind_self[c, ar, i * P + ar] = rows.astype(np.float16)

    layout = dict(cntl=cntl.astype(int), cnth=cnth.astype(int),
                  posl=posl, posh=posh, grp_cntl=grp_cntl.astype(int),
                  grp_cnth=grp_cnth.astype(int), C=C, n_groups=n_groups,
                  gstart=gstart, zfrow=zfrow)
    data = dict(eidx=eidx, ind_edge=ind_edge, ind_self=ind_self)
    return layout, data


def prepare_labels(edge_label_index, n_label, zfrow):
    """Bucket labels by (a<HALF, b<HALF) per core, pad to 128 multiples."""
    a = zfrow[np.asarray(edge_label_index[0], dtype=np.int64)]
    b = zfrow[np.asarray(edge_label_index[1], dtype=np.int64)]
    per = n_label // N_CORES
    buckets = []
    for c in range(N_CORES):
        la = a[c * per:(c + 1) * per]
        lb = b[c * per:(c + 1) * per]
        lab = np.arange(c * per, (c + 1) * per)
        bid = (la >= HALF) * 2 + (lb >= HALF)
        buckets.append([(la[bid == k], lb[bid == k], lab[bid == k])
                        for k in range(4)])
    tcnt = [max(int(np.ceil(len(buckets[c][k][0]) / P))
                for c in range(N_CORES)) for k in range(4)]
    T = sum(tcnt)
    aidx = np.zeros((N_CORES, T * P), np.int64)
    bidx = np.zeros((N_CORES, T * P), np.int64)
    labmap = np.full((N_CORES, T * P), -1, np.int64)
    for c in range(N_CORES):
        pos = 0
        for k in range(4):
            la, lb, lab = buckets[c][k]
            n = len(la)
            aidx[c, pos:pos + n] = la - (HALF if k >= 2 else 0)
            bidx[c, pos:pos + n] = lb - (HALF if k % 2 else 0)
            labmap[c, pos:pos + n] = lab
            pos += tcnt[k] * P
    la_s = np.stack([_wrap16(aidx[c]) for c in range(N_CORES)])
    lb_s = np.stack([_wrap16(bidx[c]) for c in range(N_CORES)])
    return dict(la=la_s, lb=lb_s, tcnt=tcnt, T=T, labmap=labmap)


# ------------------------------------------------------------- device kernel

def build_bass(lay, tcnt, in_c, hid_c, out_c):
    from concourse import bacc, bass, mybir
    import concourse.tile as tile

    C = lay["C"]
    T = int(sum(tcnt))
    n_groups = lay["n_groups"]
    gstart = lay["gstart"]
    cntl, cnth = lay["cntl"], lay["cnth"]
    posl, posh = lay["posl"], lay["posh"]
    grp_cntl, grp_cnth = lay["grp_cntl"], lay["grp_cnth"]
    f32 = mybir.dt.float32
    f16 = mybir.dt.float16
    i16 = mybir.dt.int16

    nc = bacc.Bacc("TRN2", target_bir_lowering=False, debug=False,
                   num_devices=N_CORES, num_swdge_queues=4)

    xf_d = nc.dram_tensor("xf", [N_NODES_PAD, in_c], f16, kind="ExternalInput")
    xs_d = nc.dram_tensor("xs", [BPC * P, in_c], f16, kind="ExternalInput")
    w_d = [nc.dram_tensor(f"W{i+1}", s, f16, kind="ExternalInput")
           for i, s in enumerate([[in_c, hid_c], [hid_c, hid_c],
                                  [hid_c, out_c]])]
    b_d = [nc.dram_tensor(f"b{i+1}", [s], f16, kind="ExternalInput")
           for i, s in enumerate([hid_c, hid_c, out_c])]
    eidx_d = nc.dram_tensor("eidx", [P, C * 8], i16, kind="ExternalInput")
    inde_d = nc.dram_tensor("inde", [P, C * P], f16, kind="ExternalInput")
    inds_d = nc.dram_tensor("inds", [P, BPC * P], f16, kind="ExternalInput")
    la_d = nc.dram_tensor("la", [P, T * 8], i16, kind="ExternalInput")
    lb_d = nc.dram_tensor("lb", [P, T * 8], i16, kind="ExternalInput")
    out_d = nc.dram_tensor("out", [P, T], f32, kind="ExternalOutput")

    zs_d = [nc.dram_tensor(f"zs{l}", [BPC * P, w], dt, kind="Internal")
            for l, (w, dt) in enumerate([(hid_c, f16), (hid_c, f16),
                                         (out_c, f32)])]
    zf_d = [nc.dram_tensor(f"zf{l}", [N_NODES_PAD, w], dt, kind="Internal",
                           addr_space="Shared")
            for l, (w, dt) in enumerate([(hid_c, f16), (hid_c, f16),
                                         (out_c, f32)])]

    gq = [0]

    def next_q():
        q = gq[0]
        gq[0] = (q + 1) % 4
        return q

    with tile.TileContext(nc) as tc:
        with (
            tc.tile_pool(name="consts", bufs=1) as cst,
            tc.tile_pool(name="gath", bufs=3) as gp,
            tc.tile_pool(name="indp", bufs=2) as ip,
            tc.tile_pool(name="selfg", bufs=4) as sp,
            tc.tile_pool(name="outp", bufs=4) as op,
            tc.tile_pool(name="dec", bufs=2) as dp,
            tc.tile_pool(name="psA", bufs=4, space="PSUM") as psA,
            tc.tile_pool(name="psZ", bufs=2, space="PSUM") as psZ,
        ):
            # ---- resident constants
            ones1 = cst.tile([1, P], f16)
            nc.vector.memset(ones1[:], 1.0)

            eidx_sb = cst.tile([P, C * 8], i16)
            nc.sync.dma_start(eidx_sb[:], eidx_d[:, :])
            inds_sb = cst.tile([P, BPC * P], f16)
            nc.sync.dma_start(inds_sb[:], inds_d[:, :])
            la_sb = cst.tile([P, T * 8], i16)
            nc.sync.dma_start(la_sb[:], la_d[:, :])
            lb_sb = cst.tile([P, T * 8], i16)
            nc.sync.dma_start(lb_sb[:], lb_d[:, :])

            w_sb, bias_sb = [], []
            for l in range(3):
                wt = cst.tile([hid_c if l else in_c,
                               out_c if l == 2 else hid_c], f16)
                nc.sync.dma_start(wt[:], w_d[l][:, :])
                w_sb.append(wt)
                bt = cst.tile([1, out_c if l == 2 else hid_c], f16)
                nc.sync.dma_start(bt[:], b_d[l][None, :])
                bias_sb.append(bt)

            # ---- 3 GCN layers
            for l in range(3):
                oc = out_c if l == 2 else hid_c
                ztype = f32 if l == 2 else f16
                lo_tab = xf_d[:, :] if l == 0 else zf_d[l - 1][:, :]
                hi_tab = (xf_d[HALF:, :] if l == 0
                          else zf_d[l - 1][HALF:, :])
                prev = xs_d if l == 0 else zs_d[l - 1]

                for g in range(n_groups):
                    b0, b1 = int(gstart[g]), int(gstart[g + 1])
                    gcl, gch = int(grp_cntl[g]), int(grp_cnth[g])
                    gcnt = gcl + gch
                    gbase = int(posl[b0])
                    gt = gp.tile([P, gcnt * in_c], f16, tag="gath")
                    g3 = gt[:].rearrange("p (c f) -> p c f", c=gcnt)
                    # split into ~4 calls on distinct queues: SWDGE
                    # descriptor emission parallelizes across queues
                    bounds = sorted({0, (gcnt + 3) // 4, gcnt // 2,
                                     3 * gcnt // 4, gcl, gcnt})
                    for a, b in zip(bounds, bounds[1:]):
                        if b <= a:
                            continue
                        tab = lo_tab if b <= gcl else hi_tab
                        nc.gpsimd.dma_gather(
                            out_ap=g3[:, a:b, :],
                            in_ap=tab,
                            idxs_ap=eidx_sb[:, (gbase + a) * 8:
                                            (gbase + b) * 8],
                            num_idxs=(b - a) * P, num_idxs_reg=(b - a) * P,
                            elem_size=in_c, single_packet=True,
                            queue_num=next_q())

                    # streamed indicators for this group's chunks
                    it = ip.tile([P, gcnt * P], f16, tag="ind")
                    nc.sync.dma_start(
                        it[:], inde_d[:, gbase * P:(gbase + gcnt) * P])

                    for i in range(b0, b1):
                        cl, ch = int(cntl[i]), int(cnth[i])
                        selfg = sp.tile([P, in_c], f16, tag="selfg")
                        nc.sync.dma_start(selfg[:],
                                          prev[i * P:(i + 1) * P, :])
                        agg_ps = psA.tile([P, P], f32, tag="agg",
                                          space="PSUM")
                        nc.tensor.matmul(
                            out=agg_ps[:], lhsT=selfg[:],
                            rhs=inds_sb[:, i * P:(i + 1) * P],
                            start=True, stop=(cl + ch == 0))
                        for which, cnt, pos0 in ((0, cl, int(posl[i])),
                                                 (1, ch, int(posh[i]))):
                            for k in range(cnt):
                                ck = pos0 + k
                                gk = ck - gbase
                                last = (which == 1 or ch == 0) and \
                                       (k == cnt - 1)
                                nc.tensor.matmul(
                                    out=agg_ps[:], lhsT=g3[:, gk, :],
                                    rhs=it[:, gk * P:(gk + 1) * P],
                                    start=False, stop=last)

                        aggT = op.tile([P, P], f16, tag="aggT")
                        nc.scalar.copy(out=aggT[:], in_=agg_ps[:])

                        z_ps = psZ.tile([P, oc], f32, tag="z", space="PSUM")
                        nc.tensor.matmul(out=z_ps[:], lhsT=ones1[:],
                                         rhs=bias_sb[l][:],
                                         start=True, stop=False)
                        nc.tensor.matmul(out=z_ps[:], lhsT=aggT[:],
                                         rhs=w_sb[l][:],
                                         start=False, stop=True)

                        z_sb = op.tile([P, oc], ztype, tag="z_sb")
                        if l < 2:
                            nc.vector.tensor_scalar_max(
                                out=z_sb[:], in0=z_ps[:], scalar1=0.0)
                        else:
                            nc.vector.tensor_copy(out=z_sb[:], in_=z_ps[:])
                        nc.sync.dma_start(zs_d[l][i * P:(i + 1) * P, :],
                                          z_sb[:])

                    if b1 == SEG0_BLOCKS:
                        # segment A complete: AG it while B computes
                        nc.gpsimd.collective_compute(
                            "AllGather", mybir.AluOpType.bypass,
                            replica_groups=[list(range(N_CORES))],
                            ins=[zs_d[l][:SEG0_BLOCKS * P, :]],
                            outs=[zf_d[l][:N_CORES * SEG0_BLOCKS * P, :]])
                nc.gpsimd.collective_compute(
                    "AllGather", mybir.AluOpType.bypass,
                    replica_groups=[list(range(N_CORES))],
                    ins=[zs_d[l][SEG0_BLOCKS * P:, :]],
                    outs=[zf_d[l][N_CORES * SEG0_BLOCKS * P:, :]])

            # ---- decode (pieces of 32 chunks to bound SBUF)
            z3 = zf_d[2]
            res = cst.tile([P, T], f32)
            tbase = 0
            for k in range(4):
                tk = int(tcnt[k])
                if tk == 0:
                    continue
                a_tab = z3[HALF:, :] if k >= 2 else z3[:, :]
                b_tab = z3[HALF:, :] if k % 2 else z3[:, :]
                for h0 in range(0, tk, 24):
                    hk = min(24, tk - h0)
                    ga = dp.tile([P, 24 * out_c], f32, tag="ga")
                    gb = dp.tile([P, 24 * out_c], f32, tag="gb")
                    ga3 = ga[:, :hk * out_c].rearrange(
                        "p (c f) -> p c f", c=hk)
                    gb3 = gb[:, :hk * out_c].rearrange(
                        "p (c f) -> p c f", c=hk)
                    t0 = tbase + h0
                    nc.gpsimd.dma_gather(
                        out_ap=ga3, in_ap=a_tab,
                        idxs_ap=la_sb[:, t0 * 8:(t0 + hk) * 8],
                        num_idxs=hk * P, num_idxs_reg=hk * P,
                        elem_size=out_c, single_packet=True,
                        queue_num=next_q())
                    nc.gpsimd.dma_gather(
                        out_ap=gb3, in_ap=b_tab,
                        idxs_ap=lb_sb[:, t0 * 8:(t0 + hk) * 8],
                        num_idxs=hk * P, num_idxs_reg=hk * P,
                        elem_size=out_c, single_packet=True,
                        queue_num=next_q())
                    nc.vector.tensor_mul(out=ga[:, :hk * out_c],
                                         in0=ga[:, :hk * out_c],
                                         in1=gb[:, :hk * out_c])
                    nc.vector.tensor_reduce(
                        out=res[:, t0:t0 + hk], in_=ga3,
                        axis=mybir.AxisListType.X, op=mybir.AluOpType.add)
                tbase += tk
            nc.sync.dma_start(out_d[:, :], res[:])

    nc.finalize()
    return nc


# ---------------------------------------------------------------- entry point

def kernel(x, W1, b1, W2, b2, W3, b3, edge_index, edge_label_index):
    from concourse.bass_utils import run_bass_kernel_spmd

    x = np.asarray(x, dtype=np.float32)
    n_nodes, in_c = x.shape
    hid_c = np.asarray(W2).shape[0]
    out_c = np.asarray(W3).shape[1]
    n_label = np.asarray(edge_label_index).shape[1]

    lay, data = prepare(edge_index, n_nodes)
    lb = prepare_labels(edge_label_index, n_label, lay["zfrow"])

    nc = build_bass(lay, lb["tcnt"], in_c, hid_c, out_c)

    xf = np.zeros((N_NODES_PAD, in_c), np.float16)
    xf[lay["zfrow"][:n_nodes]] = x[:n_nodes].astype(np.float16)

    common = {
        "xf": xf,
        "W1": np.asarray(W1).astype(np.float16),
        "W2": np.asarray(W2).astype(np.float16),
        "W3": np.asarray(W3).astype(np.float16),
        "b1": np.asarray(b1).astype(np.float16),
        "b2": np.asarray(b2).astype(np.float16),
        "b3": np.asarray(b3).astype(np.float16),
    }
    in_maps = []
    for c in range(N_CORES):
        m = dict(common)
        S0 = SEG0_BLOCKS * P
        m["xs"] = np.ascontiguousarray(np.concatenate([
            xf[c * S0:(c + 1) * S0],
            xf[N_CORES * S0 + c * (BPC * P - S0):
               N_CORES * S0 + (c + 1) * (BPC * P - S0)]]))
        m["eidx"] = np.ascontiguousarray(data["eidx"][c])
        m["inde"] = np.ascontiguousarray(data["ind_edge"][c])
        m["inds"] = np.ascontiguousarray(data["ind_self"][c])
        m["la"] = np.ascontiguousarray(lb["la"][c])
        m["lb"] = np.ascontiguousarray(lb["lb"][c])
        in_maps.append(m)

    res = run_bass_kernel_spmd(nc, in_maps, core_ids=list(range(N_CORES)))

    out = np.zeros((n_label,), np.float32)
    for c in range(N_CORES):
        o = res.results[c]["out"]  # [P, T]
        flat = o.T.reshape(-1)
        lm = lb["labmap"][c]
        valid = lm >= 0
        out[lm[valid]] = flat[valid]
    return out


# revision 9
# speedup vs baseline: 1.8526x; 1.0378x over previous
"""GCN message-passing kernel for Trainium2, 8 NeuronCores (v6).

Math (reference): 3-layer GCN with symmetric normalization and self-loops,
then dot-product decode over label edge pairs.

Reformulation: A_hat @ (x @ W) == (A_hat @ x) @ W, so each layer is
  agg = A_hat @ z          (sparse gather + PE indicator-matmul scatter)
  z   = relu(agg @ W + b)
A_hat is shared by all 3 layers; normalization folded into per-edge values.

v6 design:
- fp16 data path; z3 + decode fp32. Indicators precomputed host-side and
  streamed from DRAM (HWDGE), zero per-chunk DVE work.
- The critical resource is GpSimd Q7 descriptor emission (~3-5ns/desc,
  serial). Everything is organized to keep it busy continuously:
  * z is published in TWO segments (A = each core's first 30 blocks,
    B = rest) via two AllGathers into SEPARATE shared tensors zfA/zfB.
    AG_A fires mid-layer; the NEXT layer's A-chunk gathers depend only
    on zfA, so their descriptor emission overlaps the current layer's
    tail compute and AG_B.
  * chunks are classed by source segment (A: rows < 30720, B: rest) -
    both segments are int16-addressable relative to their table base,
    replacing the old 32768 low/high split.
- Per-block source dedup: an edge source appearing k times for one
  (block, class) occupies ONE gather slot; its indicator column has k
  nonzeros.
- dst blocks assigned to cores snake-wise by edge count so the SPMD
  program's per-block chunk counts (max over cores) have ~zero padding.
- Self-loops via contiguous HWDGE dma_start from the core's own local
  zs slice + resident diag(dinv^2) indicator (start=True clears psum).
- Decode bucketed by (a-seg, b-seg); the (A,A) bucket is gathered right
  after AG3_A, overlapping layer-3 tail.
"""

import numpy as np

P = 128
N_CORES = 8
BPC = 49                # blocks per core (392 / 8)
GROUPS = [6] * 8 + [1]  # blocks per gather group (sum = 49)
SEG0_BLOCKS = 30        # segment A blocks per core
SA = SEG0_BLOCKS * P            # 3840 local rows in segment A
SB = BPC * P - SA               # 2432 local rows in segment B
NA = N_CORES * SA               # 30720 zf rows in segment A
NB = N_CORES * SB               # 19456 zf rows in segment B
N_NODES_PAD = NA + NB


def _zf_base(core, lrow):
    """zf row of (core, local row) under the segment-major layout."""
    if lrow < SA:
        return core * SA + lrow
    return NA + core * SB + (lrow - SA)


def _wrap16(flat_idx):
    """dma_gather idx layout: idx i at [i%16, i//16], replicated to 128 rows."""
    t = flat_idx.astype(np.int16).reshape(-1, 16).T
    return np.tile(t, (8, 1))


# ---------------------------------------------------------------- host prep

def prepare(edge_index, n_nodes):
    src = np.asarray(edge_index[0], dtype=np.int64)
    dst = np.asarray(edge_index[1], dtype=np.int64)
    deg = np.bincount(dst, minlength=N_NODES_PAD).astype(np.float64) + 1.0
    dinv = (1.0 / np.sqrt(deg)).astype(np.float32)

    n_blocks = N_CORES * BPC
    blk_edges = np.bincount(dst >> 7, minlength=n_blocks)

    order = np.argsort(-blk_edges, kind="stable")
    blk_core = np.empty(n_blocks, np.int64)
    blk_local = np.empty(n_blocks, np.int64)
    for r, b in enumerate(order):
        rnd, pos = divmod(r, N_CORES)
        core = pos if rnd % 2 == 0 else N_CORES - 1 - pos
        blk_core[b] = core
        blk_local[b] = rnd
    lrow = blk_local * P
    seg_start = np.where(lrow < SA,
                         blk_core * SA + lrow,
                         NA + blk_core * SB + (lrow - SA))
    zfrow = (seg_start[np.arange(N_NODES_PAD) >> 7]
             + (np.arange(N_NODES_PAD) & 127))

    srow = zfrow[src]
    drow = zfrow[dst]
    enrm = (dinv[src] * dinv[dst]).astype(np.float32)

    dblk = drow >> 7
    dnl = (drow & 127).astype(np.int32)
    cls = (srow >= NA).astype(np.int64)   # 0 = segment A source, 1 = B

    # dedup: one slot per (block, class, source row)
    so = np.lexsort((srow, cls, dblk))
    srow_s, dnl_s, enrm_s = srow[so], dnl[so], enrm[so]
    dblk_s, cls_s = dblk[so], cls[so]
    newslot = np.ones(len(so), bool)
    newslot[1:] = ((srow_s[1:] != srow_s[:-1]) | (cls_s[1:] != cls_s[:-1])
                   | (dblk_s[1:] != dblk_s[:-1]))
    slot_id = np.cumsum(newslot) - 1          # per-edge unique-slot index
    n_slots = int(slot_id[-1]) + 1
    slot_srow = srow_s[newslot]
    slot_blk = dblk_s[newslot]
    slot_cls = cls_s[newslot]

    # per (block, class) slot counts
    key = slot_blk * 2 + slot_cls
    cnts = np.bincount(key, minlength=2 * n_blocks)
    B_of = np.empty((N_CORES, BPC), np.int64)
    for c in range(N_CORES):
        for i in range(BPC):
            B_of[c, i] = _zf_base(c, i * P) >> 7
    nA = cnts[0::2][B_of]
    nB = cnts[1::2][B_of]
    cntA = np.ceil(nA / P).astype(np.int64).max(axis=0)
    cntB = np.ceil(nB / P).astype(np.int64).max(axis=0)

    C = int((cntA + cntB).sum())
    gidx = np.zeros((N_CORES, C * P), np.int64)

    n_groups = len(GROUPS)
    gstart = np.cumsum([0] + GROUPS)
    posA = np.zeros(BPC, np.int64)
    posB = np.zeros(BPC, np.int64)
    pos = 0
    grp_cntA = np.zeros(n_groups, np.int64)
    grp_cntB = np.zeros(n_groups, np.int64)
    for g in range(n_groups):
        b0, b1 = int(gstart[g]), int(gstart[g + 1])
        for i in range(b0, b1):
            posA[i] = pos
            pos += cntA[i]
        for i in range(b0, b1):
            posB[i] = pos
            pos += cntB[i]
        grp_cntA[g] = cntA[b0:b1].sum()
        grp_cntB[g] = cntB[b0:b1].sum()
    assert pos == C

    # within-(block,class) ordinal of each unique slot
    first_of_key = np.zeros(2 * n_blocks, np.int64)
    kstart = np.searchsorted(key, np.arange(2 * n_blocks))
    within = np.arange(n_slots) - kstart[key]

    ind_edge = np.zeros((N_CORES, P, C * P), np.float16)
    for c in range(N_CORES):
        spos = np.full(2 * n_blocks, -1, np.int64)
        for i in range(BPC):
            b = B_of[c, i]
            spos[2 * b] = posA[i] * P
            spos[2 * b + 1] = posB[i] * P
        slot_pos = np.where(spos[key] >= 0, spos[key] + within, -1)
        m = slot_pos >= 0
        gidx[c, slot_pos[m]] = slot_srow[m] - slot_cls[m] * NA
        es = slot_pos[slot_id]            # per sorted-edge stream position
        ev = es >= 0
        epos = es[ev]
        np.add.at(
            ind_edge[c],
            (epos % P, (epos // P) * P + dnl_s[ev]),
            enrm_s[ev].astype(np.float16))

    eidx = np.stack([_wrap16(gidx[c]) for c in range(N_CORES)])

    # self-loop diag indicators
    dinv2 = (dinv * dinv).astype(np.float32)
    node_of_row = np.empty(N_NODES_PAD, np.int64)
    node_of_row[zfrow] = np.arange(N_NODES_PAD)
    dinv2_row = dinv2[node_of_row]
    ind_self = np.zeros((N_CORES, P, BPC * P), np.float16)
    ar = np.arange(P)
    for c in range(N_CORES):
        for i in range(BPC):
            r0 = _zf_base(c, i * P)
            ind_self[c, ar, i * P + ar] = \
                dinv2_row[r0:r0 + P].astype(np.float16)

    layout = dict(cntA=cntA.astype(int), cntB=cntB.astype(int),
                  posA=posA, posB=posB, grp_cntA=grp_cntA.astype(int),
                  grp_cntB=grp_cntB.astype(int), C=C, n_groups=n_groups,
                  gstart=gstart, zfrow=zfrow)
    data = dict(eidx=eidx, ind_edge=ind_edge, ind_self=ind_self)
    return layout, data


def prepare_labels(edge_label_index, n_label, zfrow):
    """Bucket labels by (a-seg, b-seg) per core; (A,A) bucket first."""
    a = zfrow[np.asarray(edge_label_index[0], dtype=np.int64)]
    b = zfrow[np.asarray(edge_label_index[1], dtype=np.int64)]
    per = n_label // N_CORES
    buckets = []
    for c in range(N_CORES):
        la = a[c * per:(c + 1) * per]
        lb = b[c * per:(c + 1) * per]
        lab = np.arange(c * per, (c + 1) * per)
        bid = (la >= NA) * 2 + (lb >= NA)
        buckets.append([(la[bid == k], lb[bid == k], lab[bid == k])
                        for k in range(4)])
    tcnt = [max(int(np.ceil(len(buckets[c][k][0]) / P))
                for c in range(N_CORES)) for k in range(4)]
    T = sum(tcnt)
    aidx = np.zeros((N_CORES, T * P), np.int64)
    bidx = np.zeros((N_CORES, T * P), np.int64)
    labmap = np.full((N_CORES, T * P), -1, np.int64)
    for c in range(N_CORES):
        pos = 0
        for k in range(4):
            la, lb, lab = buckets[c][k]
            n = len(la)
            aidx[c, pos:pos + n] = la - (NA if k >= 2 else 0)
            bidx[c, pos:pos + n] = lb - (NA if k % 2 else 0)
            labmap[c, pos:pos + n] = lab
            pos += tcnt[k] * P
    la_s = np.stack([_wrap16(aidx[c]) for c in range(N_CORES)])
    lb_s = np.stack([_wrap16(bidx[c]) for c in range(N_CORES)])
    return dict(la=la_s, lb=lb_s, tcnt=tcnt, T=T, labmap=labmap)


# ------------------------------------------------------------- device kernel

def build_bass(lay, tcnt, in_c, hid_c, out_c):
    from concourse import bacc, bass, mybir
    import concourse.tile as tile

    C = lay["C"]
    T = int(sum(tcnt))
    n_groups = lay["n_groups"]
    gstart = lay["gstart"]
    cntA, cntB = lay["cntA"], lay["cntB"]
    posA, posB = lay["posA"], lay["posB"]
    grp_cntA, grp_cntB = lay["grp_cntA"], lay["grp_cntB"]
    f32 = mybir.dt.float32
    f16 = mybir.dt.float16
    i16 = mybir.dt.int16

    nc = bacc.Bacc("TRN2", target_bir_lowering=False, debug=False,
                   num_devices=N_CORES, num_swdge_queues=4)

    xf_d = nc.dram_tensor("xf", [N_NODES_PAD, in_c], f16,
                          kind="ExternalInput")
    xs_d = nc.dram_tensor("xs", [BPC * P, in_c], f16, kind="ExternalInput")
    w_d = [nc.dram_tensor(f"W{i+1}", s, f16, kind="ExternalInput")
           for i, s in enumerate([[in_c, hid_c], [hid_c, hid_c],
                                  [hid_c, out_c]])]
    b_d = [nc.dram_tensor(f"b{i+1}", [s], f16, kind="ExternalInput")
           for i, s in enumerate([hid_c, hid_c, out_c])]
    eidx_d = nc.dram_tensor("eidx", [P, C * 8], i16, kind="ExternalInput")
    inde_d = nc.dram_tensor("inde", [P, C * P], f16, kind="ExternalInput")
    inds_d = nc.dram_tensor("inds", [P, BPC * P], f16, kind="ExternalInput")
    la_d = nc.dram_tensor("la", [P, T * 8], i16, kind="ExternalInput")
    lb_d = nc.dram_tensor("lb", [P, T * 8], i16, kind="ExternalInput")
    out_d = nc.dram_tensor("out", [P, T], f32, kind="ExternalOutput")

    zs_d = [nc.dram_tensor(f"zs{l}", [BPC * P, w], dt, kind="Internal")
            for l, (w, dt) in enumerate([(hid_c, f16), (hid_c, f16),
                                         (out_c, f32)])]
    zfA_d = [nc.dram_tensor(f"zfA{l}", [NA, w], dt, kind="Internal",
                            addr_space="Shared")
             for l, (w, dt) in enumerate([(hid_c, f16), (hid_c, f16),
                                          (out_c, f32)])]
    zfB_d = [nc.dram_tensor(f"zfB{l}", [NB, w], dt, kind="Internal",
                            addr_space="Shared")
             for l, (w, dt) in enumerate([(hid_c, f16), (hid_c, f16),
                                          (out_c, f32)])]

    gq = [0]

    def next_q():
        q = gq[0]
        gq[0] = (q + 1) % 4
        return q

    rg = [list(range(N_CORES))]

    with tile.TileContext(nc) as tc:
        with (
            tc.tile_pool(name="consts", bufs=1) as cst,
            tc.tile_pool(name="gathA", bufs=3) as gpa,
            tc.tile_pool(name="gathB", bufs=2) as gpb,
            tc.tile_pool(name="indp", bufs=2) as ip,
            tc.tile_pool(name="selfg", bufs=4) as sp,
            tc.tile_pool(name="outp", bufs=4) as op,
            tc.tile_pool(name="dec", bufs=2) as dp,
            tc.tile_pool(name="psA", bufs=4, space="PSUM") as psA,
            tc.tile_pool(name="psZ", bufs=2, space="PSUM") as psZ,
        ):
            # ---- resident constants
            ones1 = cst.tile([1, P], f16)
            nc.vector.memset(ones1[:], 1.0)

            eidx_sb = cst.tile([P, C * 8], i16)
            nc.sync.dma_start(eidx_sb[:], eidx_d[:, :])
            inds_sb = cst.tile([P, BPC * P], f16)
            nc.sync.dma_start(inds_sb[:], inds_d[:, :])
            la_sb = cst.tile([P, T * 8], i16)
            nc.sync.dma_start(la_sb[:], la_d[:, :])
            lb_sb = cst.tile([P, T * 8], i16)
            nc.sync.dma_start(lb_sb[:], lb_d[:, :])

            w_sb, bias_sb = [], []
            for l in range(3):
                wt = cst.tile([hid_c if l else in_c,
                               out_c if l == 2 else hid_c], f16)
                nc.sync.dma_start(wt[:], w_d[l][:, :])
                w_sb.append(wt)
                bt = cst.tile([1, out_c if l == 2 else hid_c], f16)
                nc.sync.dma_start(bt[:], b_d[l][None, :])
                bias_sb.append(bt)

            def gather_calls(g3s, tab, c0, c1, nsplit):
                n = c1 - c0
                if n <= 0:
                    return
                step = (n + nsplit - 1) // nsplit
                for a in range(c0, c1, step):
                    b = min(a + step, c1)
                    nc.gpsimd.dma_gather(
                        out_ap=g3s[:, a - c0:b - c0, :],
                        in_ap=tab,
                        idxs_ap=eidx_sb[:, a * 8:b * 8],
                        num_idxs=(b - a) * P, num_idxs_reg=(b - a) * P,
                        elem_size=in_c, single_packet=False,
                        queue_num=next_q())

            # ---- 3 GCN layers
            for l in range(3):
                oc = out_c if l == 2 else hid_c
                ztype = f32 if l == 2 else f16
                A_tab = xf_d[:NA, :] if l == 0 else zfA_d[l - 1][:, :]
                B_tab = xf_d[NA:, :] if l == 0 else zfB_d[l - 1][:, :]
                prev = xs_d if l == 0 else zs_d[l - 1]

                for g in range(n_groups):
                    b0, b1 = int(gstart[g]), int(gstart[g + 1])
                    gcA, gcB = int(grp_cntA[g]), int(grp_cntB[g])
                    baseA = int(posA[b0])
                    baseB = int(posB[b0])
                    gta = gpa.tile([P, max(gcA, 1) * in_c], f16, tag="ga")
                    g3a = gta[:].rearrange("p (c f) -> p c f",
                                           c=max(gcA, 1))
                    gtb = gpb.tile([P, max(gcB, 1) * in_c], f16, tag="gb")
                    g3b = gtb[:].rearrange("p (c f) -> p c f",
                                           c=max(gcB, 1))
                    gather_calls(g3a, A_tab, baseA, baseA + gcA, 2)
                    gather_calls(g3b, B_tab, baseB, baseB + gcB, 2)

                    it = ip.tile([P, (gcA + gcB) * P], f16, tag="ind")
                    nc.sync.dma_start(
                        it[:], inde_d[:, baseA * P:(baseA + gcA + gcB) * P])

                    for i in range(b0, b1):
                        cA, cB = int(cntA[i]), int(cntB[i])
                        selfg = sp.tile([P, in_c], f16, tag="selfg")
                        nc.sync.dma_start(selfg[:],
                                          prev[i * P:(i + 1) * P, :])
                        agg_ps = psA.tile([P, P], f32, tag="agg",
                                          space="PSUM")
                        nc.tensor.matmul(
                            out=agg_ps[:], lhsT=selfg[:],
                            rhs=inds_sb[:, i * P:(i + 1) * P],
                            start=True, stop=(cA + cB == 0))
                        for which, cnt, pos0, g3, cbase in (
                                (0, cA, int(posA[i]), g3a, baseA),
                                (1, cB, int(posB[i]), g3b, baseB)):
                            for k in range(cnt):
                                ck = pos0 + k
                                last = (which == 1 or cB == 0) and \
                                       (k == cnt - 1)
                                nc.tensor.matmul(
                                    out=agg_ps[:],
                                    lhsT=g3[:, ck - cbase, :],
                                    rhs=it[:, (ck - baseA) * P:
                                           (ck - baseA + 1) * P],
                                    start=False, stop=last)

                        aggT = op.tile([P, P], f16, tag="aggT")
                        nc.scalar.copy(out=aggT[:], in_=agg_ps[:])

                        z_ps = psZ.tile([P, oc], f32, tag="z",
                                        space="PSUM")
                        nc.tensor.matmul(out=z_ps[:], lhsT=ones1[:],
                                         rhs=bias_sb[l][:],
                                         start=True, stop=False)
                        nc.tensor.matmul(out=z_ps[:], lhsT=aggT[:],
                                         rhs=w_sb[l][:],
                                         start=False, stop=True)

                        z_sb = op.tile([P, oc], ztype, tag="z_sb")
                        if l < 2:
                            nc.vector.tensor_scalar_max(
                                out=z_sb[:], in0=z_ps[:], scalar1=0.0)
                        else:
                            nc.vector.tensor_copy(out=z_sb[:],
                                                  in_=z_ps[:])
                        nc.sync.dma_start(zs_d[l][i * P:(i + 1) * P, :],
                                          z_sb[:])

                    if b1 == SEG0_BLOCKS:
                        nc.gpsimd.collective_compute(
                            "AllGather", mybir.AluOpType.bypass,
                            replica_groups=rg,
                            ins=[zs_d[l][:SA, :]], outs=[zfA_d[l][:, :]])
                nc.gpsimd.collective_compute(
                    "AllGather", mybir.AluOpType.bypass,
                    replica_groups=rg,
                    ins=[zs_d[l][SA:, :]], outs=[zfB_d[l][:, :]])

            # ---- decode; bucket 0 = (A,A) depends only on zfA
            res = cst.tile([P, T], f32)
            tbase = 0
            for k in range(4):
                tk = int(tcnt[k])
                if tk == 0:
                    continue
                a_tab = zfB_d[2][:, :] if k >= 2 else zfA_d[2][:, :]
                b_tab = zfB_d[2][:, :] if k % 2 else zfA_d[2][:, :]
                for h0 in range(0, tk, 24):
                    hk = min(24, tk - h0)
                    ga = dp.tile([P, 24 * out_c], f32, tag="dga")
                    gb = dp.tile([P, 24 * out_c], f32, tag="dgb")
                    ga3 = ga[:, :hk * out_c].rearrange(
                        "p (c f) -> p c f", c=hk)
                    gb3 = gb[:, :hk * out_c].rearrange(
                        "p (c f) -> p c f", c=hk)
                    t0 = tbase + h0
                    nc.gpsimd.dma_gather(
                        out_ap=ga3, in_ap=a_tab,
                        idxs_ap=la_sb[:, t0 * 8:(t0 + hk) * 8],
                        num_idxs=hk * P, num_idxs_reg=hk * P,
                        elem_size=out_c, single_packet=False,
                        queue_num=next_q())
                    nc.gpsimd.dma_gather(
                        out_ap=gb3, in_ap=b_tab,
                        idxs_ap=lb_sb[:, t0 * 8:(t0 + hk) * 8],
                        num_idxs=hk * P, num_idxs_reg=hk * P,
                        elem_size=out_c, single_packet=False,
                        queue_num=next_q())
                    nc.vector.tensor_mul(out=ga[:, :hk * out_c],
                                         in0=ga[:, :hk * out_c],
                                         in1=gb[:, :hk * out_c])
                    nc.vector.tensor_reduce(
                        out=res[:, t0:t0 + hk], in_=ga3,
                        axis=mybir.AxisListType.X, op=mybir.AluOpType.add)
                tbase += tk
            nc.sync.dma_start(out_d[:, :], res[:])

    nc.finalize()
    return nc


# ---------------------------------------------------------------- entry point

def kernel(x, W1, b1, W2, b2, W3, b3, edge_index, edge_label_index):
    from concourse.bass_utils import run_bass_kernel_spmd

    x = np.asarray(x, dtype=np.float32)
    n_nodes, in_c = x.shape
    hid_c = np.asarray(W2).shape[0]
    out_c = np.asarray(W3).shape[1]
    n_label = np.asarray(edge_label_index).shape[1]

    lay, data = prepare(edge_index, n_nodes)
    lb = prepare_labels(edge_label_index, n_label, lay["zfrow"])

    nc = build_bass(lay, lb["tcnt"], in_c, hid_c, out_c)

    xf = np.zeros((N_NODES_PAD, in_c), np.float16)
    xf[lay["zfrow"][:n_nodes]] = x[:n_nodes].astype(np.float16)

    common = {
        "xf": xf,
        "W1": np.asarray(W1).astype(np.float16),
        "W2": np.asarray(W2).astype(np.float16),
        "W3": np.asarray(W3).astype(np.float16),
        "b1": np.asarray(b1).astype(np.float16),
        "b2": np.asarray(b2).astype(np.float16),
        "b3": np.asarray(b3).astype(np.float16),
    }
    in_maps = []
    for c in range(N_CORES):
        m = dict(common)
        m["xs"] = np.ascontiguousarray(np.concatenate([
            xf[c * SA:(c + 1) * SA],
            xf[NA + c * SB:NA + (c + 1) * SB]]))
        m["eidx"] = np.ascontiguousarray(data["eidx"][c])
        m["inde"] = np.ascontiguousarray(data["ind_edge"][c])
        m["inds"] = np.ascontiguousarray(data["ind_self"][c])
        m["la"] = np.ascontiguousarray(lb["la"][c])
        m["lb"] = np.ascontiguousarray(lb["lb"][c])
        in_maps.append(m)

    res = run_bass_kernel_spmd(nc, in_maps, core_ids=list(range(N_CORES)))

    out = np.zeros((n_label,), np.float32)
    for c in range(N_CORES):
        o = res.results[c]["out"]  # [P, T]
        flat = o.T.reshape(-1)
        lm = lb["labmap"][c]
        valid = lm >= 0
        out[lm[valid]] = flat[valid]
    return out


# revision 10
# speedup vs baseline: 1.8721x; 1.0105x over previous
"""GCN message-passing kernel for Trainium2, 8 NeuronCores (v6).

Math (reference): 3-layer GCN with symmetric normalization and self-loops,
then dot-product decode over label edge pairs.

Reformulation: A_hat @ (x @ W) == (A_hat @ x) @ W, so each layer is
  agg = A_hat @ z          (sparse gather + PE indicator-matmul scatter)
  z   = relu(agg @ W + b)
A_hat is shared by all 3 layers; normalization folded into per-edge values.

v6 design:
- fp16 data path; z3 + decode fp32. Indicators precomputed host-side and
  streamed from DRAM (HWDGE), zero per-chunk DVE work.
- The critical resource is GpSimd Q7 descriptor emission (~3-5ns/desc,
  serial). Everything is organized to keep it busy continuously:
  * z is published in TWO segments (A = each core's first 30 blocks,
    B = rest) via two AllGathers into SEPARATE shared tensors zfA/zfB.
    AG_A fires mid-layer; the NEXT layer's A-chunk gathers depend only
    on zfA, so their descriptor emission overlaps the current layer's
    tail compute and AG_B.
  * chunks are classed by source segment (A: rows < 30720, B: rest) -
    both segments are int16-addressable relative to their table base,
    replacing the old 32768 low/high split.
- Per-block source dedup: an edge source appearing k times for one
  (block, class) occupies ONE gather slot; its indicator column has k
  nonzeros.
- dst blocks assigned to cores snake-wise by edge count so the SPMD
  program's per-block chunk counts (max over cores) have ~zero padding.
- Self-loops via contiguous HWDGE dma_start from the core's own local
  zs slice + resident diag(dinv^2) indicator (start=True clears psum).
- Decode bucketed by (a-seg, b-seg); the (A,A) bucket is gathered right
  after AG3_A, overlapping layer-3 tail.
"""

import numpy as np

P = 128
N_CORES = 8
BPC = 49                # blocks per core (392 / 8)
GROUPS = [5] * 9 + [4]  # blocks per gather group (sum = 49)
SEG0_BLOCKS = 30        # segment A blocks per core
SA = SEG0_BLOCKS * P            # 3840 local rows in segment A
SB = BPC * P - SA               # 2432 local rows in segment B
NA = N_CORES * SA               # 30720 zf rows in segment A
NB = N_CORES * SB               # 19456 zf rows in segment B
N_NODES_PAD = NA + NB


def _zf_base(core, lrow):
    """zf row of (core, local row) under the segment-major layout."""
    if lrow < SA:
        return core * SA + lrow
    return NA + core * SB + (lrow - SA)


def _wrap16(flat_idx):
    """dma_gather idx layout: idx i at [i%16, i//16], replicated to 128 rows."""
    t = flat_idx.astype(np.int16).reshape(-1, 16).T
    return np.tile(t, (8, 1))


# ---------------------------------------------------------------- host prep

def prepare(edge_index, n_nodes):
    src = np.asarray(edge_index[0], dtype=np.int64)
    dst = np.asarray(edge_index[1], dtype=np.int64)
    deg = np.bincount(dst, minlength=N_NODES_PAD).astype(np.float64) + 1.0
    dinv = (1.0 / np.sqrt(deg)).astype(np.float32)

    n_blocks = N_CORES * BPC
    blk_edges = np.bincount(dst >> 7, minlength=n_blocks)

    order = np.argsort(-blk_edges, kind="stable")
    blk_core = np.empty(n_blocks, np.int64)
    blk_local = np.empty(n_blocks, np.int64)
    for r, b in enumerate(order):
        rnd, pos = divmod(r, N_CORES)
        core = pos if rnd % 2 == 0 else N_CORES - 1 - pos
        blk_core[b] = core
        blk_local[b] = rnd
    lrow = blk_local * P
    seg_start = np.where(lrow < SA,
                         blk_core * SA + lrow,
                         NA + blk_core * SB + (lrow - SA))
    zfrow = (seg_start[np.arange(N_NODES_PAD) >> 7]
             + (np.arange(N_NODES_PAD) & 127))

    srow = zfrow[src]
    drow = zfrow[dst]
    enrm = (dinv[src] * dinv[dst]).astype(np.float32)

    dblk = drow >> 7
    dnl = (drow & 127).astype(np.int32)
    cls = (srow >= NA).astype(np.int64)   # 0 = segment A source, 1 = B

    # dedup: one slot per (block, class, source row)
    so = np.lexsort((srow, cls, dblk))
    srow_s, dnl_s, enrm_s = srow[so], dnl[so], enrm[so]
    dblk_s, cls_s = dblk[so], cls[so]
    newslot = np.ones(len(so), bool)
    newslot[1:] = ((srow_s[1:] != srow_s[:-1]) | (cls_s[1:] != cls_s[:-1])
                   | (dblk_s[1:] != dblk_s[:-1]))
    slot_id = np.cumsum(newslot) - 1          # per-edge unique-slot index
    n_slots = int(slot_id[-1]) + 1
    slot_srow = srow_s[newslot]
    slot_blk = dblk_s[newslot]
    slot_cls = cls_s[newslot]

    # per (block, class) slot counts
    key = slot_blk * 2 + slot_cls
    cnts = np.bincount(key, minlength=2 * n_blocks)
    B_of = np.empty((N_CORES, BPC), np.int64)
    for c in range(N_CORES):
        for i in range(BPC):
            B_of[c, i] = _zf_base(c, i * P) >> 7
    nA = cnts[0::2][B_of]
    nB = cnts[1::2][B_of]
    cntA = np.ceil(nA / P).astype(np.int64).max(axis=0)
    cntB = np.ceil(nB / P).astype(np.int64).max(axis=0)

    C = int((cntA + cntB).sum())
    gidx = np.zeros((N_CORES, C * P), np.int64)

    n_groups = len(GROUPS)
    gstart = np.cumsum([0] + GROUPS)
    posA = np.zeros(BPC, np.int64)
    posB = np.zeros(BPC, np.int64)
    pos = 0
    grp_cntA = np.zeros(n_groups, np.int64)
    grp_cntB = np.zeros(n_groups, np.int64)
    for g in range(n_groups):
        b0, b1 = int(gstart[g]), int(gstart[g + 1])
        for i in range(b0, b1):
            posA[i] = pos
            pos += cntA[i]
        for i in range(b0, b1):
            posB[i] = pos
            pos += cntB[i]
        grp_cntA[g] = cntA[b0:b1].sum()
        grp_cntB[g] = cntB[b0:b1].sum()
    assert pos == C

    # within-(block,class) ordinal of each unique slot
    first_of_key = np.zeros(2 * n_blocks, np.int64)
    kstart = np.searchsorted(key, np.arange(2 * n_blocks))
    within = np.arange(n_slots) - kstart[key]

    ind_edge = np.zeros((N_CORES, P, C * P), np.float16)
    for c in range(N_CORES):
        spos = np.full(2 * n_blocks, -1, np.int64)
        for i in range(BPC):
            b = B_of[c, i]
            spos[2 * b] = posA[i] * P
            spos[2 * b + 1] = posB[i] * P
        slot_pos = np.where(spos[key] >= 0, spos[key] + within, -1)
        m = slot_pos >= 0
        gidx[c, slot_pos[m]] = slot_srow[m] - slot_cls[m] * NA
        es = slot_pos[slot_id]            # per sorted-edge stream position
        ev = es >= 0
        epos = es[ev]
        np.add.at(
            ind_edge[c],
            (epos % P, (epos // P) * P + dnl_s[ev]),
            enrm_s[ev].astype(np.float16))

    eidx = np.stack([_wrap16(gidx[c]) for c in range(N_CORES)])

    # self-loop diag indicators
    dinv2 = (dinv * dinv).astype(np.float32)
    node_of_row = np.empty(N_NODES_PAD, np.int64)
    node_of_row[zfrow] = np.arange(N_NODES_PAD)
    dinv2_row = dinv2[node_of_row]
    ind_self = np.zeros((N_CORES, P, BPC * P), np.float16)
    ar = np.arange(P)
    for c in range(N_CORES):
        for i in range(BPC):
            r0 = _zf_base(c, i * P)
            ind_self[c, ar, i * P + ar] = \
                dinv2_row[r0:r0 + P].astype(np.float16)

    layout = dict(cntA=cntA.astype(int), cntB=cntB.astype(int),
                  posA=posA, posB=posB, grp_cntA=grp_cntA.astype(int),
                  grp_cntB=grp_cntB.astype(int), C=C, n_groups=n_groups,
                  gstart=gstart, zfrow=zfrow)
    data = dict(eidx=eidx, ind_edge=ind_edge, ind_self=ind_self)
    return layout, data


def prepare_labels(edge_label_index, n_label, zfrow):
    """Bucket labels by (a-seg, b-seg) per core; (A,A) bucket first."""
    a = zfrow[np.asarray(edge_label_index[0], dtype=np.int64)]
    b = zfrow[np.asarray(edge_label_index[1], dtype=np.int64)]
    per = n_label // N_CORES
    buckets = []
    for c in range(N_CORES):
        la = a[c * per:(c + 1) * per]
        lb = b[c * per:(c + 1) * per]
        lab = np.arange(c * per, (c + 1) * per)
        bid = (la >= NA) * 2 + (lb >= NA)
        buckets.append([(la[bid == k], lb[bid == k], lab[bid == k])
                        for k in range(4)])
    tcnt = [max(int(np.ceil(len(buckets[c][k][0]) / P))
                for c in range(N_CORES)) for k in range(4)]
    T = sum(tcnt)
    aidx = np.zeros((N_CORES, T * P), np.int64)
    bidx = np.zeros((N_CORES, T * P), np.int64)
    labmap = np.full((N_CORES, T * P), -1, np.int64)
    for c in range(N_CORES):
        pos = 0
        for k in range(4):
            la, lb, lab = buckets[c][k]
            n = len(la)
            aidx[c, pos:pos + n] = la - (NA if k >= 2 else 0)
            bidx[c, pos:pos + n] = lb - (NA if k % 2 else 0)
            labmap[c, pos:pos + n] = lab
            pos += tcnt[k] * P
    la_s = np.stack([_wrap16(aidx[c]) for c in range(N_CORES)])
    lb_s = np.stack([_wrap16(bidx[c]) for c in range(N_CORES)])
    return dict(la=la_s, lb=lb_s, tcnt=tcnt, T=T, labmap=labmap)


# ------------------------------------------------------------- device kernel

def build_bass(lay, tcnt, in_c, hid_c, out_c):
    from concourse import bacc, bass, mybir
    import concourse.tile as tile

    C = lay["C"]
    T = int(sum(tcnt))
    n_groups = lay["n_groups"]
    gstart = lay["gstart"]
    cntA, cntB = lay["cntA"], lay["cntB"]
    posA, posB = lay["posA"], lay["posB"]
    grp_cntA, grp_cntB = lay["grp_cntA"], lay["grp_cntB"]
    f32 = mybir.dt.float32
    f16 = mybir.dt.float16
    i16 = mybir.dt.int16

    nc = bacc.Bacc("TRN2", target_bir_lowering=False, debug=False,
                   num_devices=N_CORES, num_swdge_queues=4)

    xf_d = nc.dram_tensor("xf", [N_NODES_PAD, in_c], f16,
                          kind="ExternalInput")
    xs_d = nc.dram_tensor("xs", [BPC * P, in_c], f16, kind="ExternalInput")
    w_d = [nc.dram_tensor(f"W{i+1}", s, f16, kind="ExternalInput")
           for i, s in enumerate([[in_c, hid_c], [hid_c, hid_c],
                                  [hid_c, out_c]])]
    b_d = [nc.dram_tensor(f"b{i+1}", [s], f16, kind="ExternalInput")
           for i, s in enumerate([hid_c, hid_c, out_c])]
    eidx_d = nc.dram_tensor("eidx", [P, C * 8], i16, kind="ExternalInput")
    inde_d = nc.dram_tensor("inde", [P, C * P], f16, kind="ExternalInput")
    inds_d = nc.dram_tensor("inds", [P, BPC * P], f16, kind="ExternalInput")
    la_d = nc.dram_tensor("la", [P, T * 8], i16, kind="ExternalInput")
    lb_d = nc.dram_tensor("lb", [P, T * 8], i16, kind="ExternalInput")
    out_d = nc.dram_tensor("out", [P, T], f32, kind="ExternalOutput")

    zs_d = [nc.dram_tensor(f"zs{l}", [BPC * P, w], dt, kind="Internal")
            for l, (w, dt) in enumerate([(hid_c, f16), (hid_c, f16),
                                         (out_c, f32)])]
    zfA_d = [nc.dram_tensor(f"zfA{l}", [NA, w], dt, kind="Internal",
                            addr_space="Shared")
             for l, (w, dt) in enumerate([(hid_c, f16), (hid_c, f16),
                                          (out_c, f32)])]
    zfB_d = [nc.dram_tensor(f"zfB{l}", [NB, w], dt, kind="Internal",
                            addr_space="Shared")
             for l, (w, dt) in enumerate([(hid_c, f16), (hid_c, f16),
                                          (out_c, f32)])]

    gq = [0]

    def next_q():
        q = gq[0]
        gq[0] = (q + 1) % 4
        return q

    rg = [list(range(N_CORES))]

    with tile.TileContext(nc) as tc:
        with (
            tc.tile_pool(name="consts", bufs=1) as cst,
            tc.tile_pool(name="gathA", bufs=4) as gpa,
            tc.tile_pool(name="gathB", bufs=3) as gpb,
            tc.tile_pool(name="indp", bufs=2) as ip,
            tc.tile_pool(name="selfg", bufs=4) as sp,
            tc.tile_pool(name="outp", bufs=4) as op,
            tc.tile_pool(name="dec", bufs=2) as dp,
            tc.tile_pool(name="psA", bufs=4, space="PSUM") as psA,
            tc.tile_pool(name="psZ", bufs=2, space="PSUM") as psZ,
        ):
            # ---- resident constants
            ones1 = cst.tile([1, P], f16)
            nc.vector.memset(ones1[:], 1.0)

            eidx_sb = cst.tile([P, C * 8], i16)
            nc.sync.dma_start(eidx_sb[:], eidx_d[:, :])
            inds_sb = cst.tile([P, BPC * P], f16)
            nc.sync.dma_start(inds_sb[:], inds_d[:, :])
            la_sb = cst.tile([P, T * 8], i16)
            nc.sync.dma_start(la_sb[:], la_d[:, :])
            lb_sb = cst.tile([P, T * 8], i16)
            nc.sync.dma_start(lb_sb[:], lb_d[:, :])

            w_sb, bias_sb = [], []
            for l in range(3):
                wt = cst.tile([hid_c if l else in_c,
                               out_c if l == 2 else hid_c], f16)
                nc.sync.dma_start(wt[:], w_d[l][:, :])
                w_sb.append(wt)
                bt = cst.tile([1, out_c if l == 2 else hid_c], f16)
                nc.sync.dma_start(bt[:], b_d[l][None, :])
                bias_sb.append(bt)

            def gather_calls(g3s, tab, c0, c1, nsplit):
                n = c1 - c0
                if n <= 0:
                    return
                step = (n + nsplit - 1) // nsplit
                for a in range(c0, c1, step):
                    b = min(a + step, c1)
                    nc.gpsimd.dma_gather(
                        out_ap=g3s[:, a - c0:b - c0, :],
                        in_ap=tab,
                        idxs_ap=eidx_sb[:, a * 8:b * 8],
                        num_idxs=(b - a) * P, num_idxs_reg=(b - a) * P,
                        elem_size=in_c, single_packet=False,
                        queue_num=next_q())

            # ---- 3 GCN layers
            for l in range(3):
                oc = out_c if l == 2 else hid_c
                ztype = f32 if l == 2 else f16
                A_tab = xf_d[:NA, :] if l == 0 else zfA_d[l - 1][:, :]
                B_tab = xf_d[NA:, :] if l == 0 else zfB_d[l - 1][:, :]
                prev = xs_d if l == 0 else zs_d[l - 1]

                g3a_q, g3b_q, it_q = {}, {}, {}

                def issue_A(g):
                    if g >= n_groups:
                        return
                    b0 = int(gstart[g])
                    gcA = int(grp_cntA[g])
                    baseA = int(posA[b0])
                    gta = gpa.tile([P, max(gcA, 1) * in_c], f16, tag="ga",
                                   name=f"ga{l}_{g}")
                    g3a = gta[:].rearrange("p (c f) -> p c f",
                                           c=max(gcA, 1))
                    gather_calls(g3a, A_tab, baseA, baseA + gcA, 2)
                    g3a_q[g] = g3a

                def issue_B(g):
                    if g >= n_groups:
                        return
                    b0 = int(gstart[g])
                    gcB = int(grp_cntB[g])
                    baseB = int(posB[b0])
                    gtb = gpb.tile([P, max(gcB, 1) * in_c], f16, tag="gb",
                                   name=f"gb{l}_{g}")
                    g3b = gtb[:].rearrange("p (c f) -> p c f",
                                           c=max(gcB, 1))
                    gather_calls(g3b, B_tab, baseB, baseB + gcB, 2)
                    g3b_q[g] = g3b

                def issue_ind(g):
                    if g >= n_groups:
                        return
                    b0 = int(gstart[g])
                    gcA, gcB = int(grp_cntA[g]), int(grp_cntB[g])
                    baseA = int(posA[b0])
                    it = ip.tile([P, (gcA + gcB) * P], f16, tag="ind",
                                 name=f"it{l}_{g}")
                    nc.sync.dma_start(
                        it[:], inde_d[:, baseA * P:(baseA + gcA + gcB) * P])
                    it_q[g] = it

                issue_A(0)
                issue_A(1)
                issue_A(2)
                issue_B(0)
                issue_B(1)
                issue_ind(0)
                issue_ind(1)

                for g in range(n_groups):
                    b0, b1 = int(gstart[g]), int(gstart[g + 1])
                    baseA = int(posA[b0])
                    baseB = int(posB[b0])
                    issue_A(g + 3)
                    issue_B(g + 2)
                    issue_ind(g + 2)
                    g3a, g3b, it = g3a_q.pop(g), g3b_q.pop(g), it_q.pop(g)

                    for i in range(b0, b1):
                        cA, cB = int(cntA[i]), int(cntB[i])
                        selfg = sp.tile([P, in_c], f16, tag="selfg")
                        nc.sync.dma_start(selfg[:],
                                          prev[i * P:(i + 1) * P, :])
                        agg_ps = psA.tile([P, P], f32, tag="agg",
                                          space="PSUM")
                        nc.tensor.matmul(
                            out=agg_ps[:], lhsT=selfg[:],
                            rhs=inds_sb[:, i * P:(i + 1) * P],
                            start=True, stop=(cA + cB == 0))
                        for which, cnt, pos0, g3, cbase in (
                                (0, cA, int(posA[i]), g3a, baseA),
                                (1, cB, int(posB[i]), g3b, baseB)):
                            for k in range(cnt):
                                ck = pos0 + k
                                last = (which == 1 or cB == 0) and \
                                       (k == cnt - 1)
                                nc.tensor.matmul(
                                    out=agg_ps[:],
                                    lhsT=g3[:, ck - cbase, :],
                                    rhs=it[:, (ck - baseA) * P:
                                           (ck - baseA + 1) * P],
                                    start=False, stop=last)

                        aggT = op.tile([P, P], f16, tag="aggT")
                        nc.scalar.copy(out=aggT[:], in_=agg_ps[:])

                        z_ps = psZ.tile([P, oc], f32, tag="z",
                                        space="PSUM")
                        nc.tensor.matmul(out=z_ps[:], lhsT=ones1[:],
                                         rhs=bias_sb[l][:],
                                         start=True, stop=False)
                        nc.tensor.matmul(out=z_ps[:], lhsT=aggT[:],
                                         rhs=w_sb[l][:],
                                         start=False, stop=True)

                        z_sb = op.tile([P, oc], ztype, tag="z_sb")
                        if l < 2:
                            nc.vector.tensor_scalar_max(
                                out=z_sb[:], in0=z_ps[:], scalar1=0.0)
                        else:
                            nc.vector.tensor_copy(out=z_sb[:],
                                                  in_=z_ps[:])
                        nc.sync.dma_start(zs_d[l][i * P:(i + 1) * P, :],
                                          z_sb[:])

                    if b1 == SEG0_BLOCKS:
                        nc.gpsimd.collective_compute(
                            "AllGather", mybir.AluOpType.bypass,
                            replica_groups=rg,
                            ins=[zs_d[l][:SA, :]], outs=[zfA_d[l][:, :]])
                nc.gpsimd.collective_compute(
                    "AllGather", mybir.AluOpType.bypass,
                    replica_groups=rg,
                    ins=[zs_d[l][SA:, :]], outs=[zfB_d[l][:, :]])

            # ---- decode; bucket 0 = (A,A) depends only on zfA
            res = cst.tile([P, T], f32)
            tbase = 0
            for k in range(4):
                tk = int(tcnt[k])
                if tk == 0:
                    continue
                a_tab = zfB_d[2][:, :] if k >= 2 else zfA_d[2][:, :]
                b_tab = zfB_d[2][:, :] if k % 2 else zfA_d[2][:, :]
                for h0 in range(0, tk, 16):
                    hk = min(16, tk - h0)
                    ga = dp.tile([P, 16 * out_c], f32, tag="dga")
                    gb = dp.tile([P, 16 * out_c], f32, tag="dgb")
                    ga3 = ga[:, :hk * out_c].rearrange(
                        "p (c f) -> p c f", c=hk)
                    gb3 = gb[:, :hk * out_c].rearrange(
                        "p (c f) -> p c f", c=hk)
                    t0 = tbase + h0
                    nc.gpsimd.dma_gather(
                        out_ap=ga3, in_ap=a_tab,
                        idxs_ap=la_sb[:, t0 * 8:(t0 + hk) * 8],
                        num_idxs=hk * P, num_idxs_reg=hk * P,
                        elem_size=out_c, single_packet=False,
                        queue_num=next_q())
                    nc.gpsimd.dma_gather(
                        out_ap=gb3, in_ap=b_tab,
                        idxs_ap=lb_sb[:, t0 * 8:(t0 + hk) * 8],
                        num_idxs=hk * P, num_idxs_reg=hk * P,
                        elem_size=out_c, single_packet=False,
                        queue_num=next_q())
                    nc.vector.tensor_mul(out=ga[:, :hk * out_c],
                                         in0=ga[:, :hk * out_c],
                                         in1=gb[:, :hk * out_c])
                    nc.vector.tensor_reduce(
                        out=res[:, t0:t0 + hk], in_=ga3,
                        axis=mybir.AxisListType.X, op=mybir.AluOpType.add)
                tbase += tk
            nc.sync.dma_start(out_d[:, :], res[:])

    nc.finalize()
    return nc


# ---------------------------------------------------------------- entry point

def kernel(x, W1, b1, W2, b2, W3, b3, edge_index, edge_label_index):
    from concourse.bass_utils import run_bass_kernel_spmd

    x = np.asarray(x, dtype=np.float32)
    n_nodes, in_c = x.shape
    hid_c = np.asarray(W2).shape[0]
    out_c = np.asarray(W3).shape[1]
    n_label = np.asarray(edge_label_index).shape[1]

    lay, data = prepare(edge_index, n_nodes)
    lb = prepare_labels(edge_label_index, n_label, lay["zfrow"])

    nc = build_bass(lay, lb["tcnt"], in_c, hid_c, out_c)

    xf = np.zeros((N_NODES_PAD, in_c), np.float16)
    xf[lay["zfrow"][:n_nodes]] = x[:n_nodes].astype(np.float16)

    common = {
        "xf": xf,
        "W1": np.asarray(W1).astype(np.float16),
        "W2": np.asarray(W2).astype(np.float16),
        "W3": np.asarray(W3).astype(np.float16),
        "b1": np.asarray(b1).astype(np.float16),
        "b2": np.asarray(b2).astype(np.float16),
        "b3": np.asarray(b3).astype(np.float16),
    }
    in_maps = []
    for c in range(N_CORES):
        m = dict(common)
        m["xs"] = np.ascontiguousarray(np.concatenate([
            xf[c * SA:(c + 1) * SA],
            xf[NA + c * SB:NA + (c + 1) * SB]]))
        m["eidx"] = np.ascontiguousarray(data["eidx"][c])
        m["inde"] = np.ascontiguousarray(data["ind_edge"][c])
        m["inds"] = np.ascontiguousarray(data["ind_self"][c])
        m["la"] = np.ascontiguousarray(lb["la"][c])
        m["lb"] = np.ascontiguousarray(lb["lb"][c])
        in_maps.append(m)

    res = run_bass_kernel_spmd(nc, in_maps, core_ids=list(range(N_CORES)))

    out = np.zeros((n_label,), np.float32)
    for c in range(N_CORES):
        o = res.results[c]["out"]  # [P, T]
        flat = o.T.reshape(-1)
        lm = lb["labmap"][c]
        valid = lm >= 0
        out[lm[valid]] = flat[valid]
    return out
